# revision 1
# baseline (speedup 1.0000x reference)
"""Trainium2 Bass kernel for CrossAttention (B=4, L=S=2048, DIM=1024, H=16, hd=64).

Sharding: data-parallel over (batch, L-half): core c handles batch c//2,
query rows [(c%2)*1024, (c%2+1)*1024).  Each core computes the QKV
projections for its slice (K/V duplicated within a batch pair), per-head
RMSNorm, masked softmax attention, and the output projection.

Device layout is feature-major ("transposed"): activations live as
[dim, tokens] so every matmul contraction dim is on SBUF partitions with
no on-device transposes.  The host pre-transposes q/kv and casts to bf16.

Softmax: after RMS norm |score| <= 8, so no running max is needed.  exp
runs on ACT with a fused per-partition scale (k-norm rsqrt / 8) and bias
(padding mask, -1e5 -> exp == 0).  The denominator comes from a 65th
"ones" column appended to V; the division is deferred: o^T is stored
unnormalized, denominators are collected per head, and one batched
reciprocal + rank-1 PE broadcast normalizes o^T before proj_o.

Perf notes: score matmuls for a head pair run concurrently in the PE
array via tile_position row packing (K=64 each); PV and sumsq matmuls
are emitted one iteration late so the PE queue head never waits on ACT;
weight loads are shared across output halves (k-outer loops); the
normalization sweep is batched at the end, off the attention pipeline.
The projection phases are PE-bound, the attention phase is bound by the
ACT engine's exp throughput (1 elem/lane/cycle).
"""

import sys

if "/opt/trn_rl_repo" not in sys.path:
    sys.path.insert(0, "/opt/trn_rl_repo")

import numpy as np
import ml_dtypes

import concourse.bass as bass
import concourse.bacc as bacc
import concourse.tile as tile
from concourse import mybir
from concourse.bass_utils import run_bass_kernel_spmd

BF16 = ml_dtypes.bfloat16

B, L, S, DIM = 4, 2048, 2048, 1024
H, HD = 16, 64
N_CORES = 8
LC = L // 2          # query rows per core
KC = DIM // 128      # 128-partition chunks of DIM
EPS = 1e-5
MASK_BIAS = -1.0e5   # exp(-1e5) == 0 in fp32

TRACE = False        # set by test.py for profiling
LAST_RESULT = {}     # exec_time_ns etc. for test.py

_CACHE = {}


def _build(n_sc):
    """Build the SPMD Bass program; n_sc = number of 128-wide kv chunks."""
    fp32 = mybir.dt.float32
    bf16 = mybir.dt.bfloat16
    AF = mybir.ActivationFunctionType

    nc = bacc.Bacc("TRN2", target_bir_lowering=False, debug=False,
                   num_devices=N_CORES)

    qT_d = nc.dram_tensor("qT", [DIM, LC], bf16, kind="ExternalInput")
    n_half = (n_sc + 1) // 2         # kv chunks computed locally per core
    W = n_half * 128                 # local kv width
    kvT_d = nc.dram_tensor("kvT", [DIM, W], bf16, kind="ExternalInput")
    wq_d = nc.dram_tensor("wq", [DIM, DIM], bf16, kind="ExternalInput")
    wk_d = nc.dram_tensor("wk", [DIM, DIM], bf16, kind="ExternalInput")
    wv_d = nc.dram_tensor("wv", [DIM, DIM], bf16, kind="ExternalInput")
    wo_d = nc.dram_tensor("wo", [DIM, DIM], bf16, kind="ExternalInput")
    mask_d = nc.dram_tensor("mask", [128, 16], fp32, kind="ExternalInput")
    qw_d = nc.dram_tensor("qw", [128, 1], fp32, kind="ExternalInput")
    kw_d = nc.dram_tensor("kw", [128, 1], fp32, kind="ExternalInput")
    ind_d = nc.dram_tensor("ind", [KC, 128, 16], bf16, kind="ExternalInput")
    ind2_d = nc.dram_tensor("ind2", [KC, 16, 128], fp32, kind="ExternalInput")
    eye_d = nc.dram_tensor("eye16", [16, 16], fp32, kind="ExternalInput")
    out_d = nc.dram_tensor("out", [LC, DIM], fp32, kind="ExternalOutput")

    n_hg = (W + 511) // 512          # 512-wide groups over the local half

    with tile.TileContext(nc) as tc:
        with (
            tc.tile_pool(name="wp", bufs=3 * KC) as wp,          # wq wk wv (+wo reuse)
            tc.tile_pool(name="qtp", bufs=KC) as qtp,            # qT, later oT
            tc.tile_pool(name="kvp", bufs=KC) as kvp,            # kvT (+sq/exp tiles)
            tc.tile_pool(name="qhp", bufs=KC) as qhp,            # qhT
            tc.tile_pool(name="khp", bufs=KC) as khp,            # khT
            tc.tile_pool(name="vp", bufs=n_sc) as vp,            # vh_aug
            tc.tile_pool(name="sp", bufs=1) as sp,               # constants
            tc.tile_pool(name="skp", bufs=16) as skp,           # skT per chunk
            tc.tile_pool(name="tp", bufs=4) as tp,               # f32 temps
            tc.tile_pool(name="dp", bufs=1, space="DRAM") as dp,     # gather blobs
            tc.tile_pool(name="pa", bufs=2, space="PSUM") as pa,     # proj/score psum
            tc.tile_pool(name="po", bufs=4, space="PSUM") as po,     # accum/bc psum
        ):
            # ---------------- constants / inputs ----------------
            wq_sb, wk_sb, wv_sb, kvt_sb, qt_sb = [], [], [], [], []
            for k in range(KC):
                w1 = wp.tile([128, DIM], bf16, name=f"wq{k}", tag="w")
                nc.sync.dma_start(out=w1, in_=wq_d[k * 128:(k + 1) * 128, :])
                wq_sb.append(w1)
            for k in range(KC):
                w2 = wp.tile([128, DIM], bf16, name=f"wk{k}", tag="w")
                nc.sync.dma_start(out=w2, in_=wk_d[k * 128:(k + 1) * 128, :])
                wk_sb.append(w2)
            for k in range(KC):
                w3 = wp.tile([128, DIM], bf16, name=f"wv{k}", tag="w")
                nc.sync.dma_start(out=w3, in_=wv_d[k * 128:(k + 1) * 128, :])
                wv_sb.append(w3)
            for k in range(KC):
                qt = qtp.tile([128, LC], bf16, name=f"qt{k}", tag="qt")
                nc.sync.dma_start(out=qt, in_=qT_d[k * 128:(k + 1) * 128, :])
                qt_sb.append(qt)
            for k in range(KC):
                kvt = kvp.tile([128, W], bf16, name=f"kvt{k}", tag="kv")
                nc.sync.dma_start(out=kvt, in_=kvT_d[k * 128:(k + 1) * 128, :])
                kvt_sb.append(kvt)

            mask_sb = sp.tile([128, 16], fp32, name="mask")
            nc.sync.dma_start(out=mask_sb, in_=mask_d[:, :])
            qw_sb = sp.tile([128, 1], fp32, name="qw")
            nc.sync.dma_start(out=qw_sb, in_=qw_d[:, :])
            kw_sb = sp.tile([128, 1], fp32, name="kw")
            nc.sync.dma_start(out=kw_sb, in_=kw_d[:, :])
            eye_sb = sp.tile([16, 16], fp32, name="eye16")
            nc.sync.dma_start(out=eye_sb, in_=eye_d[:, :])
            ind_sb, ind2_sb = [], []
            for k in range(KC):
                i1 = sp.tile([128, 16], bf16, name=f"ind{k}")
                nc.sync.dma_start(out=i1, in_=ind_d[k, :, :])
                ind_sb.append(i1)
                i2 = sp.tile([16, 128], fp32, name=f"ind2{k}")
                nc.sync.dma_start(out=i2, in_=ind2_d[k, :, :])
                ind2_sb.append(i2)
            epsq_sb = sp.tile([16, 1], fp32, name="epsq")
            nc.vector.memset(epsq_sb, EPS)
            epsk_sb = sp.tile([16, 1], fp32, name="epsk")
            nc.vector.memset(epsk_sb, 64.0 * EPS)

            # -------- K projection + skT on the LOCAL kv half --------
            khh_sb = [khp.tile([128, W], bf16, name=f"khh{m}", tag="khh")
                      for m in range(KC)]
            skl_sb = [skp.tile([128, 16], fp32, name=f"skl{i}", tag="skl",
                               bufs=8)
                      for i in range(n_half)]
            for sg in range(n_hg):
                wdt = min(512, W - sg * 512)
                ssk = po.tile([16, 512], fp32, name="ssk", tag="po")
                pend = None
                for m in range(KC):
                    ps = pa.tile([128, 1024], fp32, name="proj_ps", tag="pa")
                    for k in range(KC):
                        nc.tensor.matmul(
                            ps[:, :wdt],
                            lhsT=wk_sb[k][:, m * 128:(m + 1) * 128],
                            rhs=kvt_sb[k][:, sg * 512:sg * 512 + wdt],
                            start=(k == 0), stop=(k == KC - 1))
                    if pend is not None:
                        pm, pq = pend
                        nc.tensor.matmul(
                            ssk[:, :wdt], lhsT=ind_sb[pm], rhs=pq[:, :wdt],
                            start=(pm == 0), stop=False)
                    nc.vector.tensor_scalar_mul(
                        khh_sb[m][:, sg * 512:sg * 512 + wdt], ps[:, :wdt],
                        kw_sb)
                    ksq = kvp.tile([128, 1024], bf16, name="sqt", tag="sq",
                                   bufs=6)
                    nc.scalar.activation(ksq[:, :wdt], ps[:, :wdt], AF.Square)
                    pend = (m, ksq)
                pm, pq = pend
                nc.tensor.matmul(ssk[:, :wdt], lhsT=ind_sb[pm],
                                 rhs=pq[:, :wdt], start=False, stop=True)
                # 8*sqrt(mean+eps) = sqrt(sumsq + 64 eps); recip -> sk/8
                skr = tp.tile([16, 512], fp32, name="skr", tag="small16", bufs=2)
                nc.scalar.activation(skr[:, :wdt], ssk[:, :wdt], AF.Sqrt,
                                     scale=1.0, bias=epsk_sb)
                for t in range(wdt // 128):
                    tpp = po.tile([128, 16], fp32, name="sktp", tag="po")
                    nc.tensor.transpose(tpp, skr[:, t * 128:(t + 1) * 128],
                                        eye_sb)
                    nc.vector.reciprocal(out=skl_sb[sg * 4 + t], in_=tpp)

            # -------- AllGather 1: kh half (overlaps the V projection) ----
            VA_W = H * 65
            TOTA = KC * 128 * W
            TOTB = n_half * 128 * (VA_W + 32)
            off_sk = n_half * 128 * VA_W
            groups = [[2 * x, 2 * x + 1] for x in range(N_CORES // 2)]
            blobA_loc = dp.tile([TOTA], bf16, name="blobA_loc")
            blobA_g = dp.tile([2 * TOTA], bf16, name="blobA_g")
            for m in range(KC):
                nc.sync.dma_start(
                    out=blobA_loc[m * 128 * W:(m + 1) * 128 * W]
                    .rearrange("(p x) -> p x", p=128),
                    in_=khh_sb[m][:, :])
            nc.gpsimd.collective_compute(
                "AllGather", mybir.AluOpType.bypass, replica_groups=groups,
                ins=[blobA_loc.opt()], outs=[blobA_g.opt()])

            # -------- V projection on the LOCAL kv half (ones-augmented) ----
            va_loc = []
            for i in range(n_half):
                va = vp.tile([128, H * 65], bf16, name=f"val{i}", tag="va",
                             bufs=n_sc + 2)
                ones_cols = bass.AP(tensor=va.tensor, offset=va.offset + 64,
                                    ap=[list(va.ap[0]), [65, H], [1, 1]])
                nc.vector.memset(ones_cols, 1.0)
                ps = pa.tile([128, 1024], fp32, name="proj_ps", tag="pa")
                for k in range(KC):          # k outer: one LDW serves both jn
                    for jn in range(2):
                        nc.tensor.matmul(
                            ps[:, jn * 512:(jn + 1) * 512],
                            lhsT=kvt_sb[k][:, i * 128:(i + 1) * 128],
                            rhs=wv_sb[k][:, jn * 512:(jn + 1) * 512],
                            start=(k == 0), stop=(k == KC - 1))
                for jn in range(2):
                    dst = bass.AP(tensor=va.tensor,
                                  offset=va.offset + 65 * 8 * jn,
                                  ap=[list(va.ap[0]), [65, 8], [1, 64]])
                    nc.vector.tensor_copy(
                        dst, ps[:, jn * 512:(jn + 1) * 512]
                        .rearrange("p (h d) -> p h d", h=8))
                va_loc.append(va)

            # -------- AllGather 2: va + sk (overlaps the Q projection) ----
            blobB_loc = dp.tile([TOTB], bf16, name="blobB_loc")
            blobB_g = dp.tile([2 * TOTB], bf16, name="blobB_g")
            for i in range(n_half):
                nc.sync.dma_start(
                    out=blobB_loc[i * 128 * VA_W:(i + 1) * 128 * VA_W]
                    .rearrange("(p x) -> p x", p=128),
                    in_=va_loc[i][:, :])
                nc.sync.dma_start(
                    out=blobB_loc[off_sk + i * 128 * 32:
                                  off_sk + (i + 1) * 128 * 32]
                    .rearrange("(p x) -> p x", p=128),
                    in_=skl_sb[i].bitcast(bf16)[:, :])
            nc.gpsimd.collective_compute(
                "AllGather", mybir.AluOpType.bypass, replica_groups=groups,
                ins=[blobB_loc.opt()], outs=[blobB_g.opt()])

            # ---------------- Q projection + q RMS stats ----------------
            # sumsq matmuls are emitted one (m, j) step late so the PE
            # queue head never blocks on the ACT Square.
            qh_sb = [qhp.tile([128, LC], bf16, name=f"qh{m}", tag="qh")
                     for m in range(KC)]
            sumsq_q = [po.tile([16, 512], fp32, name=f"ssq{j}", tag="po")
                       for j in range(2)]
            pend = None                      # (m, qsq_tile)
            for m in range(KC):
                ps = pa.tile([128, 1024], fp32, name="proj_ps", tag="pa")
                for k in range(KC):          # k outer: one LDW serves both j
                    for j in range(2):
                        nc.tensor.matmul(
                            ps[:, j * 512:(j + 1) * 512],
                            lhsT=wq_sb[k][:, m * 128:(m + 1) * 128],
                            rhs=qt_sb[k][:, j * 512:(j + 1) * 512],
                            start=(k == 0), stop=(k == KC - 1))
                if pend is not None:
                    pm, pq = pend
                    for j in range(2):
                        nc.tensor.matmul(
                            sumsq_q[j][:, :], lhsT=ind_sb[pm],
                            rhs=pq[:, j * 512:(j + 1) * 512],
                            start=(pm == 0), stop=(pm == KC - 1))
                nc.vector.tensor_scalar_mul(qh_sb[m][:, :], ps[:, :], qw_sb)
                qsq = kvp.tile([128, 1024], bf16, name="sqt", tag="sq",
                               bufs=6)
                nc.scalar.activation(qsq[:, :], ps[:, :], AF.Square)
                pend = (m, qsq)
            pm, pq = pend
            for j in range(2):
                nc.tensor.matmul(sumsq_q[j][:, :], lhsT=ind_sb[pm],
                                 rhs=pq[:, j * 512:(j + 1) * 512],
                                 start=False, stop=True)
            # sq = 1/sqrt(mean + eps); the broadcast-and-multiply sweep over
            # qhT is deferred until after the V projection so the PE stream
            # stays dense across the three projection phases.
            sq_sb = []
            for j in range(2):
                sqr = tp.tile([16, 512], fp32, name=f"sqr{j}", tag="small16", bufs=2)
                nc.scalar.activation(sqr, sumsq_q[j][:, :], AF.Sqrt,
                                     scale=1.0 / HD, bias=epsq_sb)
                sqv = tp.tile([16, 512], fp32, name=f"sqv{j}", tag="small16b", bufs=2)
                nc.vector.reciprocal(out=sqv, in_=sqr)
                sq_sb.append(sqv)
            for m in range(KC):
                for j in range(2):
                    bc = po.tile([128, 512], fp32, name="qbc", tag="po")
                    nc.tensor.matmul(bc, lhsT=ind2_sb[m], rhs=sq_sb[j],
                                     start=True, stop=True)
                    nc.vector.tensor_mul(
                        qh_sb[m][:, j * 512:(j + 1) * 512],
                        qh_sb[m][:, j * 512:(j + 1) * 512], bc)

            # -------- readback into canonical full-S tiles --------
            kh_sb = [khp.tile([128, 2 * W], bf16, name=f"kh{m}", tag="kh")
                     for m in range(KC)]
            for m in range(KC):
                for r in range(2):
                    nc.sync.dma_start(
                        out=kh_sb[m][:, r * W:(r + 1) * W],
                        in_=blobA_g[r * TOTA + m * 128 * W:
                                    r * TOTA + (m + 1) * 128 * W]
                        .rearrange("(p x) -> p x", p=128))
            va_sb, skT_sb = [], []
            for i in range(n_sc):
                r, li = i // n_half, i % n_half
                va = vp.tile([128, VA_W], bf16, name=f"va{i}", tag="va",
                             bufs=n_sc + 2)
                nc.sync.dma_start(
                    out=va[:, :],
                    in_=blobB_g[r * TOTB + li * 128 * VA_W:
                                r * TOTB + (li + 1) * 128 * VA_W]
                    .rearrange("(p x) -> p x", p=128))
                va_sb.append(va)
                sk = skp.tile([128, 16], fp32, name=f"skT{i}", tag="sk",
                              bufs=16)
                nc.sync.dma_start(
                    out=sk.bitcast(bf16)[:, :],
                    in_=blobB_g[r * TOTB + off_sk + li * 128 * 32:
                                r * TOTB + off_sk + (li + 1) * 128 * 32]
                    .rearrange("(p x) -> p x", p=128))
                skT_sb.append(sk)
            # ---------------- attention (head pairs, pipelined) ----------
            # oT holds UNNORMALIZED o^T; den_sb collects denominators.
            oT_sb = [qtp.tile([128, LC], bf16, name=f"oT{m}", tag="qt")
                     for m in range(KC)]
            den_sb = sp.tile([16, LC], fp32, name="den")
            nc.vector.memset(den_sb, 1.0)   # recip of not-yet-written rows
            for p in range(KC):              # head pair (2p, 2p+1)
                hA, hB = 2 * p, 2 * p + 1
                pv = [po.tile([128, 512], fp32, name=f"pv{x}", tag="po")
                      for x in range(4)]     # A0 A1 B0 B1
                pending = None               # (exA, exB, first)
                for i in range(n_sc):
                    scA = pa.tile([128, 1024], fp32, name="scA", tag="pa")
                    scB = pa.tile([128, 1024], fp32, name="scB", tag="pa")
                    for j in range(2):
                        nc.tensor.matmul(
                            scA[:, j * 512:(j + 1) * 512],
                            lhsT=kh_sb[p][0:64, i * 128:(i + 1) * 128],
                            rhs=qh_sb[p][0:64, j * 512:(j + 1) * 512],
                            start=True, stop=True, tile_position=(0, 0))
                        nc.tensor.matmul(
                            scB[:, j * 512:(j + 1) * 512],
                            lhsT=kh_sb[p][64:128, i * 128:(i + 1) * 128],
                            rhs=qh_sb[p][64:128, j * 512:(j + 1) * 512],
                            start=True, stop=True, tile_position=(64, 0))
                    if pending is not None:
                        exA, exB, first = pending
                        for j in range(2):
                            nc.tensor.matmul(
                                pv[j][:65, :],
                                lhsT=va_sb[i - 1][:, hA * 65:(hA + 1) * 65],
                                rhs=exA[:, j * 512:(j + 1) * 512],
                                start=first, stop=False)
                        for j in range(2):
                            nc.tensor.matmul(
                                pv[2 + j][:65, :],
                                lhsT=va_sb[i - 1][:, hB * 65:(hB + 1) * 65],
                                rhs=exB[:, j * 512:(j + 1) * 512],
                                start=first, stop=False)
                    exA = kvp.tile([128, 1024], bf16, name="exA", tag="sq",
                                   bufs=6)
                    exB = kvp.tile([128, 1024], bf16, name="exB", tag="sq",
                                   bufs=6)
                    nc.scalar.activation(exA, scA, AF.Exp,
                                         scale=skT_sb[i][:, hA:hA + 1],
                                         bias=mask_sb[:, i:i + 1])
                    nc.scalar.activation(exB, scB, AF.Exp,
                                         scale=skT_sb[i][:, hB:hB + 1],
                                         bias=mask_sb[:, i:i + 1])
                    pending = (exA, exB, i == 0)
                exA, exB, first = pending
                for j in range(2):
                    nc.tensor.matmul(
                        pv[j][:65, :],
                        lhsT=va_sb[n_sc - 1][:, hA * 65:(hA + 1) * 65],
                        rhs=exA[:, j * 512:(j + 1) * 512],
                        start=first, stop=True)
                for j in range(2):
                    nc.tensor.matmul(
                        pv[2 + j][:65, :],
                        lhsT=va_sb[n_sc - 1][:, hB * 65:(hB + 1) * 65],
                        rhs=exB[:, j * 512:(j + 1) * 512],
                        start=first, stop=True)
                # stash unnormalized o^T and the denominators (off the PE
                # path).  Engine ops need 32-aligned partition bases, so the
                # denominator row is staged at partition 64 and moved to its
                # per-head row in den_sb by a small SBUF->SBUF DMA.
                for x, (hh, j) in enumerate(((hA, 0), (hA, 1),
                                             (hB, 0), (hB, 1))):
                    poff = (hh % 2) * 64
                    nc.vector.tensor_copy(
                        oT_sb[p][poff:poff + 64, j * 512:(j + 1) * 512],
                        pv[x][0:64, :])
                    dstage = tp.tile([128, 512], fp32, name="dstage",
                                     tag="rec")
                    nc.vector.tensor_copy(dstage[64:65, :], pv[x][64:65, :])
                    nc.sync.dma_start(
                        out=den_sb[hh:hh + 1, j * 512:(j + 1) * 512],
                        in_=dstage[64:65, :])

            # ---------------- normalize o^T (batched) ----------------
            denr_sb = sp.tile([16, LC], fp32, name="denr")
            nc.vector.reciprocal(out=denr_sb, in_=den_sb)
            for m in range(KC):
                for j in range(2):
                    bc = po.tile([128, 512], fp32, name="obc", tag="po")
                    nc.tensor.matmul(bc, lhsT=ind2_sb[m],
                                     rhs=denr_sb[:, j * 512:(j + 1) * 512],
                                     start=True, stop=True)
                    nc.vector.tensor_mul(
                        oT_sb[m][:, j * 512:(j + 1) * 512],
                        oT_sb[m][:, j * 512:(j + 1) * 512], bc)

            # ---------------- output projection ----------------
            wo_sb = []
            for k in range(KC):
                w4 = wp.tile([128, DIM], bf16, name=f"wo{k}", tag="w")
                nc.sync.dma_start(out=w4, in_=wo_d[k * 128:(k + 1) * 128, :])
                wo_sb.append(w4)
            for lc in range(KC):
                for jn in range(2):
                    ps = pa.tile([128, 1024], fp32, name="proj_ps", tag="pa")
                    for k in range(KC):
                        nc.tensor.matmul(
                            ps[:, :512],
                            lhsT=oT_sb[k][:, lc * 128:(lc + 1) * 128],
                            rhs=wo_sb[k][:, jn * 512:(jn + 1) * 512],
                            start=(k == 0), stop=(k == KC - 1))
                    osb = tp.tile([128, 512], fp32, name="osb", tag="rec")
                    nc.vector.tensor_copy(osb, ps[:, :512])
                    nc.sync.dma_start(
                        out=out_d[lc * 128:(lc + 1) * 128,
                                  jn * 512:(jn + 1) * 512],
                        in_=osb)
    nc.compile()
    return nc


def kernel(**inputs):
    q = np.asarray(inputs["q"], dtype=np.float32)
    kv = np.asarray(inputs["kv"], dtype=np.float32)
    seqlens = np.asarray(inputs["x_seqlens"], dtype=np.int32)
    Wq = np.asarray(inputs["Wq"], dtype=np.float32)
    Wk = np.asarray(inputs["Wk"], dtype=np.float32)
    Wv = np.asarray(inputs["Wv"], dtype=np.float32)
    Wo = np.asarray(inputs["Wo"], dtype=np.float32)
    qnw = np.asarray(inputs["q_norm_w"], dtype=np.float32)
    knw = np.asarray(inputs["k_norm_w"], dtype=np.float32)

    n_sc = max(1, int(-(-int(seqlens.max()) // 128)))
    if n_sc not in _CACHE:
        _CACHE[n_sc] = _build(n_sc)
    nc = _CACHE[n_sc]

    wq_b = np.ascontiguousarray(Wq).astype(BF16)
    wk_b = np.ascontiguousarray(Wk).astype(BF16)
    wv_b = np.ascontiguousarray(Wv).astype(BF16)
    wo_b = np.ascontiguousarray(Wo).astype(BF16)
    qw = np.tile(qnw, 2).reshape(128, 1)
    kw = np.tile(knw, 2).reshape(128, 1)
    ind = np.zeros((KC, 128, 16), np.float32)
    ind2 = np.zeros((KC, 16, 128), np.float32)
    p = np.arange(128)
    for c in range(KC):
        ind[c, p, 2 * c + p // 64] = 1.0
        ind2[c, 2 * c + p // 64, p] = 1.0
    ind = ind.astype(BF16)
    eye16 = np.eye(16, dtype=np.float32)

    in_maps = []
    for c in range(N_CORES):
        b, half = c // 2, c % 2
        qT = np.ascontiguousarray(
            q[b, half * LC:(half + 1) * LC, :].T).astype(BF16)
        n_half = (n_sc + 1) // 2
        Wl = n_half * 128
        kvT = np.ascontiguousarray(
            kv[b].T[:, half * Wl:(half + 1) * Wl]).astype(BF16)
        sl = int(seqlens[b])
        mask = np.where(np.arange(S) < sl, 0.0, MASK_BIAS).astype(np.float32)
        mask = np.ascontiguousarray(mask.reshape(16, 128).T)
        in_maps.append({
            "qT": qT, "kvT": kvT, "wq": wq_b, "wk": wk_b, "wv": wv_b,
            "wo": wo_b, "mask": mask, "qw": qw, "kw": kw, "ind": ind,
            "ind2": ind2, "eye16": eye16,
        })

    res = run_bass_kernel_spmd(nc, in_maps, list(range(N_CORES)),
                               trace=TRACE)
    LAST_RESULT["exec_time_ns"] = res.exec_time_ns
    LAST_RESULT["profile"] = res.profile_json

    out = np.empty((B, L, DIM), np.float32)
    for c in range(N_CORES):
        b, half = c // 2, c % 2
        out[b, half * LC:(half + 1) * LC, :] = res.results[c]["out"]
    return out



# revision 4
# speedup vs baseline: 1.1958x; 1.1958x over previous
"""Trainium2 Bass kernel for CrossAttention (B=4, L=S=2048, DIM=1024, H=16, hd=64).

Sharding: data-parallel over (batch, L-half): core c handles batch c//2,
query rows [(c%2)*1024, (c%2+1)*1024).  Each core computes the QKV
projections for its slice (K/V duplicated within a batch pair), per-head
RMSNorm, masked softmax attention, and the output projection.

Device layout is feature-major ("transposed"): activations live as
[dim, tokens] so every matmul contraction dim is on SBUF partitions with
no on-device transposes.  The host pre-transposes q/kv and casts to bf16.

Softmax: after RMS norm |score| <= 8, so no running max is needed.  exp
runs on ACT with a fused per-partition scale (k-norm rsqrt / 8) and bias
(padding mask, -1e5 -> exp == 0).  The denominator comes from a 65th
"ones" column appended to V; the division is deferred: o^T is stored
unnormalized, per-head reciprocals are computed on DVE as each head
drains, and a rank-1 fp32r PE broadcast normalizes o^T right before the
output projection (interleaved with it per 512-column half).

Perf notes: input DMAs are ordered so the K projection's operands (wk,
kvT) land first and the PE starts ~12us in; wo is fetched during
attention into recycled wk slots.  The attention loop runs ONE head at a
time (score j0/j1 -> exp -> pv j0/j1, pv emitted one chunk late): per
128-pos chunk the PE owes only 4x512-col matmuls (~1.0us) vs one
[128,1024] exp on ACT (~1.4us), so ACT paces the phase and the PE never
stalls long enough to matter.  PSUM exactly fits: 2x[128,1024] score
tiles + 2x[128,1024] pv/aux tiles = 8 banks.  All rank-1 broadcast
matmuls (q-norm, o-norm) use bf16 operands (full rate) instead of
4x-slow fp32.
"""

import sys

if "/opt/trn_rl_repo" not in sys.path:
    sys.path.insert(0, "/opt/trn_rl_repo")

import numpy as np
import ml_dtypes

import concourse.bass as bass
import concourse.bacc as bacc
import concourse.tile as tile
from concourse import mybir
from concourse.bass_utils import run_bass_kernel_spmd

BF16 = ml_dtypes.bfloat16

B, L, S, DIM = 4, 2048, 2048, 1024
H, HD = 16, 64
N_CORES = 8
LC = L // 2          # query rows per core
KC = DIM // 128      # 128-partition chunks of DIM
EPS = 1e-5
MASK_BIAS = -1.0e5   # exp(-1e5) == 0 in fp32

TRACE = False        # set by test.py for profiling
LAST_RESULT = {}     # exec_time_ns etc. for test.py

_CACHE = {}


def _build(n_sc):
    """Build the SPMD Bass program; n_sc = number of 128-wide kv chunks."""
    fp32 = mybir.dt.float32
    bf16 = mybir.dt.bfloat16
    AF = mybir.ActivationFunctionType

    nc = bacc.Bacc("TRN2", target_bir_lowering=False, debug=False,
                   num_devices=N_CORES)

    qT_d = nc.dram_tensor("qT", [DIM, LC], bf16, kind="ExternalInput")
    n_half = (n_sc + 1) // 2         # kv chunks computed locally per core
    W = n_half * 128                 # local kv width
    kvT_d = nc.dram_tensor("kvT", [DIM, W], bf16, kind="ExternalInput")
    wq_d = nc.dram_tensor("wq", [DIM, DIM], bf16, kind="ExternalInput")
    wk_d = nc.dram_tensor("wk", [DIM, DIM], bf16, kind="ExternalInput")
    wv_d = nc.dram_tensor("wv", [DIM, DIM], bf16, kind="ExternalInput")
    wo_d = nc.dram_tensor("wo", [DIM, DIM], bf16, kind="ExternalInput")
    mask_d = nc.dram_tensor("mask", [128, 16], fp32, kind="ExternalInput")
    qw_d = nc.dram_tensor("qw", [128, 1], fp32, kind="ExternalInput")
    kw_d = nc.dram_tensor("kw", [128, 1], fp32, kind="ExternalInput")
    ind_d = nc.dram_tensor("ind", [KC, 128, 16], bf16, kind="ExternalInput")
    ind2_d = nc.dram_tensor("ind2", [KC, 16, 128], bf16, kind="ExternalInput")
    eye_d = nc.dram_tensor("eye16", [16, 16], fp32, kind="ExternalInput")
    out_d = nc.dram_tensor("out", [LC, DIM], fp32, kind="ExternalOutput")

    n_hg = (W + 511) // 512          # 512-wide groups over the local half

    with tile.TileContext(nc) as tc:
        with (
            tc.tile_pool(name="wp", bufs=3 * KC) as wp,          # wk wv wq (+wo reuse)
            tc.tile_pool(name="qtp", bufs=KC) as qtp,            # qT, later oT
            tc.tile_pool(name="kvp", bufs=KC) as kvp,            # kvT (+sq/exp tiles)
            tc.tile_pool(name="qhp", bufs=KC) as qhp,            # qhT
            tc.tile_pool(name="khp", bufs=KC) as khp,            # khT
            tc.tile_pool(name="vp", bufs=n_sc) as vp,            # vh_aug
            tc.tile_pool(name="sp", bufs=1) as sp,               # constants
            tc.tile_pool(name="skp", bufs=16) as skp,           # skT per chunk
            tc.tile_pool(name="tp", bufs=4) as tp,               # f32 temps
            tc.tile_pool(name="dp", bufs=1, space="DRAM") as dp,     # gather blobs
            tc.tile_pool(name="pa", bufs=2, space="PSUM") as pa,     # proj/score psum
            tc.tile_pool(name="po", bufs=2, space="PSUM") as po,     # pv/aux psum
        ):
            # ---------------- constants (small, land first) ----------------
            mask_sb = sp.tile([128, 16], fp32, name="mask")
            nc.sync.dma_start(out=mask_sb, in_=mask_d[:, :])
            qw_sb = sp.tile([128, 1], fp32, name="qw")
            nc.sync.dma_start(out=qw_sb, in_=qw_d[:, :])
            kw_sb = sp.tile([128, 1], fp32, name="kw")
            nc.sync.dma_start(out=kw_sb, in_=kw_d[:, :])
            eye_sb = sp.tile([16, 16], fp32, name="eye16")
            nc.sync.dma_start(out=eye_sb, in_=eye_d[:, :])
            ind_sb, ind2_sb = [], []
            for k in range(KC):
                i1 = sp.tile([128, 16], bf16, name=f"ind{k}")
                nc.sync.dma_start(out=i1, in_=ind_d[k, :, :])
                ind_sb.append(i1)
                i2 = sp.tile([16, 128], bf16, name=f"ind2{k}")
                nc.sync.dma_start(out=i2, in_=ind2_d[k, :, :])
                ind2_sb.append(i2)
            epsq_sb = sp.tile([16, 1], fp32, name="epsq")
            nc.vector.memset(epsq_sb, EPS)
            epsk_sb = sp.tile([16, 1], fp32, name="epsk")
            nc.vector.memset(epsk_sb, 64.0 * EPS)

            # -------- inputs: K-proj operands first, then the rest --------
            wk_sb, kvt_sb, wv_sb, wq_sb, qt_sb = [], [], [], [], []
            for k in range(KC):
                w2 = wp.tile([128, DIM], bf16, name=f"wk{k}", tag="w")
                nc.sync.dma_start(out=w2, in_=wk_d[k * 128:(k + 1) * 128, :])
                wk_sb.append(w2)
            for k in range(KC):
                kvt = kvp.tile([128, W], bf16, name=f"kvt{k}", tag="kv")
                nc.sync.dma_start(out=kvt, in_=kvT_d[k * 128:(k + 1) * 128, :])
                kvt_sb.append(kvt)
            for k in range(KC):
                w3 = wp.tile([128, DIM], bf16, name=f"wv{k}", tag="w")
                nc.sync.dma_start(out=w3, in_=wv_d[k * 128:(k + 1) * 128, :])
                wv_sb.append(w3)
            for k in range(KC):
                w1 = wp.tile([128, DIM], bf16, name=f"wq{k}", tag="w")
                nc.sync.dma_start(out=w1, in_=wq_d[k * 128:(k + 1) * 128, :])
                wq_sb.append(w1)
            for k in range(KC):
                qt = qtp.tile([128, LC], bf16, name=f"qt{k}", tag="qt")
                nc.sync.dma_start(out=qt, in_=qT_d[k * 128:(k + 1) * 128, :])
                qt_sb.append(qt)

            # -------- K projection + skT on the LOCAL kv half --------
            khh_sb = [khp.tile([128, W], bf16, name=f"khh{m}", tag="khh")
                      for m in range(KC)]
            skl_sb = [skp.tile([128, 16], fp32, name=f"skl{i}", tag="skl",
                               bufs=8)
                      for i in range(n_half)]
            for sg in range(n_hg):
                wdt = min(512, W - sg * 512)
                ssk = po.tile([16, 512], fp32, name="ssk", tag="pv")
                pend = None
                for m in range(KC):
                    ps = pa.tile([128, 1024], fp32, name="proj_ps", tag="pa")
                    for k in range(KC):
                        nc.tensor.matmul(
                            ps[:, :wdt],
                            lhsT=wk_sb[k][:, m * 128:(m + 1) * 128],
                            rhs=kvt_sb[k][:, sg * 512:sg * 512 + wdt],
                            start=(k == 0), stop=(k == KC - 1))
                    if pend is not None:
                        pm, pq = pend
                        nc.tensor.matmul(
                            ssk[:, :wdt], lhsT=ind_sb[pm], rhs=pq[:, :wdt],
                            start=(pm == 0), stop=False)
                    nc.vector.tensor_scalar_mul(
                        khh_sb[m][:, sg * 512:sg * 512 + wdt], ps[:, :wdt],
                        kw_sb)
                    ksq = kvp.tile([128, 1024], bf16, name="sqt", tag="sq",
                                   bufs=6)
                    nc.scalar.activation(ksq[:, :wdt], ps[:, :wdt], AF.Square)
                    pend = (m, ksq)
                pm, pq = pend
                nc.tensor.matmul(ssk[:, :wdt], lhsT=ind_sb[pm],
                                 rhs=pq[:, :wdt], start=False, stop=True)
                # 8*sqrt(mean+eps) = sqrt(sumsq + 64 eps); recip -> sk/8
                skr = tp.tile([16, 512], fp32, name="skr", tag="small16", bufs=2)
                nc.scalar.activation(skr[:, :wdt], ssk[:, :wdt], AF.Sqrt,
                                     scale=1.0, bias=epsk_sb)
                for t in range(wdt // 128):
                    tpp = po.tile([128, 16], fp32, name="sktp", tag="pv")
                    nc.tensor.transpose(tpp, skr[:, t * 128:(t + 1) * 128],
                                        eye_sb)
                    nc.vector.reciprocal(out=skl_sb[sg * 4 + t], in_=tpp)

            # -------- AllGather 1: kh half (overlaps the V projection) ----
            VA_W = H * 65
            TOTA = KC * 128 * W
            TOTB = n_half * 128 * (VA_W + 32)
            off_sk = n_half * 128 * VA_W
            groups = [[2 * x, 2 * x + 1] for x in range(N_CORES // 2)]
            blobA_loc = dp.tile([TOTA], bf16, name="blobA_loc")
            blobA_g = dp.tile([2 * TOTA], bf16, name="blobA_g")
            for m in range(KC):
                nc.sync.dma_start(
                    out=blobA_loc[m * 128 * W:(m + 1) * 128 * W]
                    .rearrange("(p x) -> p x", p=128),
                    in_=khh_sb[m][:, :])
            nc.gpsimd.collective_compute(
                "AllGather", mybir.AluOpType.bypass, replica_groups=groups,
                ins=[blobA_loc.opt()], outs=[blobA_g.opt()])

            # -------- V projection on the LOCAL kv half (ones-augmented) ----
            va_loc = []
            for i in range(n_half):
                va = vp.tile([128, H * 65], bf16, name=f"val{i}", tag="va",
                             bufs=n_sc + 2)
                ones_cols = bass.AP(tensor=va.tensor, offset=va.offset + 64,
                                    ap=[list(va.ap[0]), [65, H], [1, 1]])
                nc.vector.memset(ones_cols, 1.0)
                ps = pa.tile([128, 1024], fp32, name="proj_ps", tag="pa")
                for k in range(KC):          # k outer: one LDW serves both jn
                    for jn in range(2):
                        nc.tensor.matmul(
                            ps[:, jn * 512:(jn + 1) * 512],
                            lhsT=kvt_sb[k][:, i * 128:(i + 1) * 128],
                            rhs=wv_sb[k][:, jn * 512:(jn + 1) * 512],
                            start=(k == 0), stop=(k == KC - 1))
                for jn in range(2):
                    dst = bass.AP(tensor=va.tensor,
                                  offset=va.offset + 65 * 8 * jn,
                                  ap=[list(va.ap[0]), [65, 8], [1, 64]])
                    nc.vector.tensor_copy(
                        dst, ps[:, jn * 512:(jn + 1) * 512]
                        .rearrange("p (h d) -> p h d", h=8))
                va_loc.append(va)

            # -------- AllGather 2: va + sk (overlaps the Q projection) ----
            blobB_loc = dp.tile([TOTB], bf16, name="blobB_loc")
            blobB_g = dp.tile([2 * TOTB], bf16, name="blobB_g")
            for i in range(n_half):
                nc.sync.dma_start(
                    out=blobB_loc[i * 128 * VA_W:(i + 1) * 128 * VA_W]
                    .rearrange("(p x) -> p x", p=128),
                    in_=va_loc[i][:, :])
                nc.sync.dma_start(
                    out=blobB_loc[off_sk + i * 128 * 32:
                                  off_sk + (i + 1) * 128 * 32]
                    .rearrange("(p x) -> p x", p=128),
                    in_=skl_sb[i].bitcast(bf16)[:, :])
            nc.gpsimd.collective_compute(
                "AllGather", mybir.AluOpType.bypass, replica_groups=groups,
                ins=[blobB_loc.opt()], outs=[blobB_g.opt()])

            # ---------------- Q projection + q RMS stats ----------------
            # sumsq matmuls are emitted one (m, j) step late so the PE
            # queue head never blocks on the ACT Square.
            qh_sb = [qhp.tile([128, LC], bf16, name=f"qh{m}", tag="qh")
                     for m in range(KC)]
            sumsq_q = [po.tile([16, 512], fp32, name=f"ssq{j}", tag="pv")
                       for j in range(2)]
            pend = None                      # (m, qsq_tile)
            for m in range(KC):
                ps = pa.tile([128, 1024], fp32, name="proj_ps", tag="pa")
                for k in range(KC):          # k outer: one LDW serves both j
                    for j in range(2):
                        nc.tensor.matmul(
                            ps[:, j * 512:(j + 1) * 512],
                            lhsT=wq_sb[k][:, m * 128:(m + 1) * 128],
                            rhs=qt_sb[k][:, j * 512:(j + 1) * 512],
                            start=(k == 0), stop=(k == KC - 1))
                if pend is not None:
                    pm, pq = pend
                    for j in range(2):
                        nc.tensor.matmul(
                            sumsq_q[j][:, :], lhsT=ind_sb[pm],
                            rhs=pq[:, j * 512:(j + 1) * 512],
                            start=(pm == 0), stop=(pm == KC - 1))
                nc.vector.tensor_scalar_mul(qh_sb[m][:, :], ps[:, :], qw_sb)
                qsq = kvp.tile([128, 1024], bf16, name="sqt", tag="sq",
                               bufs=6)
                nc.scalar.activation(qsq, ps, AF.Square)
                pend = (m, qsq)
            pm, pq = pend
            for j in range(2):
                nc.tensor.matmul(sumsq_q[j][:, :], lhsT=ind_sb[pm],
                                 rhs=pq[:, j * 512:(j + 1) * 512],
                                 start=False, stop=True)
            # sq = 1/sqrt(mean + eps); broadcast with fp32r rank-1 matmuls
            sq_sb = []
            for j in range(2):
                sqr = tp.tile([16, 512], fp32, name=f"sqr{j}", tag="small16", bufs=2)
                nc.scalar.activation(sqr, sumsq_q[j][:, :], AF.Sqrt,
                                     scale=1.0 / HD, bias=epsq_sb)
                sqv = tp.tile([16, 512], bf16, name=f"sqv{j}", tag="small16b", bufs=2)
                with nc.allow_low_precision(reason="rank-1 rms scale, bf16 ok"):
                    nc.vector.reciprocal(out=sqv, in_=sqr)
                sq_sb.append(sqv)
            for m in range(KC):
                for j in range(2):
                    bc = po.tile([128, 512], fp32, name="qbc", tag="pv")
                    nc.tensor.matmul(bc, lhsT=ind2_sb[m], rhs=sq_sb[j],
                                     start=True, stop=True)
                    nc.vector.tensor_mul(
                        qh_sb[m][:, j * 512:(j + 1) * 512],
                        qh_sb[m][:, j * 512:(j + 1) * 512], bc)

            # wo fetch now: reuses wk slots (idle since K proj), lands
            # during attention.
            wo_sb = []
            for k in range(KC):
                w4 = wp.tile([128, DIM], bf16, name=f"wo{k}", tag="w")
                nc.sync.dma_start(out=w4, in_=wo_d[k * 128:(k + 1) * 128, :])
                wo_sb.append(w4)

            # -------- readback into canonical full-S tiles --------
            kh_sb = [khp.tile([128, 2 * W], bf16, name=f"kh{m}", tag="kh")
                     for m in range(KC)]
            for m in range(KC):
                for r in range(2):
                    nc.sync.dma_start(
                        out=kh_sb[m][:, r * W:(r + 1) * W],
                        in_=blobA_g[r * TOTA + m * 128 * W:
                                    r * TOTA + (m + 1) * 128 * W]
                        .rearrange("(p x) -> p x", p=128))
            va_sb, skT_sb = [], []
            for i in range(n_sc):
                r, li = i // n_half, i % n_half
                va = vp.tile([128, VA_W], bf16, name=f"va{i}", tag="va",
                             bufs=n_sc + 2)
                nc.sync.dma_start(
                    out=va[:, :],
                    in_=blobB_g[r * TOTB + li * 128 * VA_W:
                                r * TOTB + (li + 1) * 128 * VA_W]
                    .rearrange("(p x) -> p x", p=128))
                va_sb.append(va)
                sk = skp.tile([128, 16], fp32, name=f"skT{i}", tag="sk",
                              bufs=16)
                nc.sync.dma_start(
                    out=sk.bitcast(bf16)[:, :],
                    in_=blobB_g[r * TOTB + off_sk + li * 128 * 32:
                                r * TOTB + off_sk + (li + 1) * 128 * 32]
                    .rearrange("(p x) -> p x", p=128))
                skT_sb.append(sk)

            # ------------- attention: one head at a time, ACT-paced -------
            # Per chunk the PE owes 4 N=512 matmuls (~1.0us) vs one
            # [128,1024] exp (~1.4us) on ACT, so ACT paces and the PE keeps
            # a 2-deep score pipeline.  pv is emitted one chunk late.  o^T
            # stays unnormalized; per-head reciprocals land in denr_sb.
            oT_sb = [qtp.tile([128, LC], bf16, name=f"oT{m}", tag="qt")
                     for m in range(KC)]
            denr_sb = sp.tile([16, LC], bf16, name="denr")
            nc.vector.memset(denr_sb, 1.0)   # recip of not-yet-written rows

            def emit_pv(pend):
                ph, pi, pex, ppv, first, last = pend
                for j in range(2):
                    nc.tensor.matmul(
                        ppv[:65, j * 512:(j + 1) * 512],
                        lhsT=va_sb[pi][:, ph * 65:(ph + 1) * 65],
                        rhs=pex[:, j * 512:(j + 1) * 512],
                        start=first, stop=last)
                if last:
                    m, r = ph // 2, (ph % 2) * 64
                    nc.vector.tensor_copy(oT_sb[m][r:r + 64, :], ppv[0:64, :])
                    dstage = tp.tile([128, LC], bf16, name="dstage",
                                     tag="rec", bufs=2)
                    with nc.allow_low_precision(
                            reason="softmax denom recip, bf16 ok"):
                        nc.vector.reciprocal(out=dstage[64:65, :],
                                             in_=ppv[64:65, :])
                    nc.sync.dma_start(out=denr_sb[ph:ph + 1, :],
                                      in_=dstage[64:65, :])

            pend = None
            for h in range(H):
                m, r = h // 2, (h % 2) * 64
                pv = po.tile([128, LC], fp32, name=f"pv{h}", tag="pv")
                for i in range(n_sc):
                    sc = pa.tile([128, LC], fp32, name="sc", tag="pa")
                    for j in range(2):
                        nc.tensor.matmul(
                            sc[:, j * 512:(j + 1) * 512],
                            lhsT=kh_sb[m][r:r + 64, i * 128:(i + 1) * 128],
                            rhs=qh_sb[m][r:r + 64, j * 512:(j + 1) * 512],
                            start=True, stop=True)
                    if pend is not None:
                        emit_pv(pend)
                    ex = kvp.tile([128, LC], bf16, name="ex", tag="sq",
                                  bufs=6)
                    nc.scalar.activation(ex, sc, AF.Exp,
                                         scale=skT_sb[i][:, h:h + 1],
                                         bias=mask_sb[:, i:i + 1])
                    pend = (h, i, ex, pv, i == 0, i == n_sc - 1)
            emit_pv(pend)

            # ------- normalize o^T + output projection, interleaved -------
            for j in range(2):
                for m in range(KC):
                    obc = po.tile([128, 512], fp32, name="obc", tag="pv")
                    nc.tensor.matmul(obc, lhsT=ind2_sb[m],
                                     rhs=denr_sb[:, j * 512:(j + 1) * 512],
                                     start=True, stop=True)
                    nc.vector.tensor_mul(
                        oT_sb[m][:, j * 512:(j + 1) * 512],
                        oT_sb[m][:, j * 512:(j + 1) * 512], obc)
                for lc in range(4 * j, 4 * j + 4):
                    for jn in range(2):
                        ps = pa.tile([128, 1024], fp32, name="proj_ps",
                                     tag="pa")
                        for k in range(KC):
                            nc.tensor.matmul(
                                ps[:, :512],
                                lhsT=oT_sb[k][:, lc * 128:(lc + 1) * 128],
                                rhs=wo_sb[k][:, jn * 512:(jn + 1) * 512],
                                start=(k == 0), stop=(k == KC - 1))
                        osb = tp.tile([128, 512], fp32, name="osb", tag="rec",
                                      bufs=2)
                        nc.vector.tensor_copy(osb, ps[:, :512])
                        nc.sync.dma_start(
                            out=out_d[lc * 128:(lc + 1) * 128,
                                      jn * 512:(jn + 1) * 512],
                            in_=osb)
    nc.compile()
    return nc


def kernel(**inputs):
    q = np.asarray(inputs["q"], dtype=np.float32)
    kv = np.asarray(inputs["kv"], dtype=np.float32)
    seqlens = np.asarray(inputs["x_seqlens"], dtype=np.int32)
    Wq = np.asarray(inputs["Wq"], dtype=np.float32)
    Wk = np.asarray(inputs["Wk"], dtype=np.float32)
    Wv = np.asarray(inputs["Wv"], dtype=np.float32)
    Wo = np.asarray(inputs["Wo"], dtype=np.float32)
    qnw = np.asarray(inputs["q_norm_w"], dtype=np.float32)
    knw = np.asarray(inputs["k_norm_w"], dtype=np.float32)

    n_sc = max(1, int(-(-int(seqlens.max()) // 128)))
    if n_sc not in _CACHE:
        _CACHE[n_sc] = _build(n_sc)
    nc = _CACHE[n_sc]

    wq_b = np.ascontiguousarray(Wq).astype(BF16)
    wk_b = np.ascontiguousarray(Wk).astype(BF16)
    wv_b = np.ascontiguousarray(Wv).astype(BF16)
    wo_b = np.ascontiguousarray(Wo).astype(BF16)
    qw = np.tile(qnw, 2).reshape(128, 1)
    kw = np.tile(knw, 2).reshape(128, 1)
    ind = np.zeros((KC, 128, 16), np.float32)
    ind2 = np.zeros((KC, 16, 128), np.float32)
    p = np.arange(128)
    for c in range(KC):
        ind[c, p, 2 * c + p // 64] = 1.0
        ind2[c, 2 * c + p // 64, p] = 1.0
    ind = ind.astype(BF16)
    ind2 = ind2.astype(BF16)
    eye16 = np.eye(16, dtype=np.float32)

    in_maps = []
    for c in range(N_CORES):
        b, half = c // 2, c % 2
        qT = np.ascontiguousarray(
            q[b, half * LC:(half + 1) * LC, :].T).astype(BF16)
        n_half = (n_sc + 1) // 2
        Wl = n_half * 128
        kvT = np.ascontiguousarray(
            kv[b].T[:, half * Wl:(half + 1) * Wl]).astype(BF16)
        sl = int(seqlens[b])
        mask = np.where(np.arange(S) < sl, 0.0, MASK_BIAS).astype(np.float32)
        mask = np.ascontiguousarray(mask.reshape(16, 128).T)
        in_maps.append({
            "qT": qT, "kvT": kvT, "wq": wq_b, "wk": wk_b, "wv": wv_b,
            "wo": wo_b, "mask": mask, "qw": qw, "kw": kw, "ind": ind,
            "ind2": ind2, "eye16": eye16,
        })

    res = run_bass_kernel_spmd(nc, in_maps, list(range(N_CORES)),
                               trace=TRACE)
    LAST_RESULT["exec_time_ns"] = res.exec_time_ns
    LAST_RESULT["profile"] = res.profile_json

    out = np.empty((B, L, DIM), np.float32)
    for c in range(N_CORES):
        b, half = c // 2, c % 2
        out[b, half * LC:(half + 1) * LC, :] = res.results[c]["out"]
    return out


# revision 7
# speedup vs baseline: 1.3943x; 1.1661x over previous
"""Trainium2 Bass kernel for CrossAttention (B=4, L=S=2048, DIM=1024, H=16, hd=64).

Sharding: data-parallel over (batch, L-half): core c handles batch c//2,
query rows [(c%2)*1024, (c%2+1)*1024).  Each core computes the QKV
projections for its slice (K/V duplicated within a batch pair), per-head
RMSNorm, masked softmax attention, and the output projection.

Device layout is feature-major ("transposed"): activations live as
[dim, tokens] so every matmul contraction dim is on SBUF partitions with
no on-device transposes.  The host pre-transposes q/kv and casts to bf16.

Softmax: after RMS norm |score| <= 8, so no running max is needed.  The
k-norm rsqrt/8 is folded into kh right after the K projection (rank-2 PE
broadcast via the head-indicator matmul), and the padding mask is folded
into V by zeroing masked va rows (masked exps are finite but contribute
nothing, including to the denominator's ones-column) -- the
attention-phase exp is therefore a BARE activation with no scale/bias AP
reads (~850ns vs ~1400ns per [128,1024] tile).  The denominator comes
from a 65th "ones" column appended to V; the division is deferred: o^T
is stored unnormalized, den rows are collected per head, one batched
reciprocal runs at the end, and a rank-1 bf16 PE broadcast normalizes
o^T interleaved with the output projection per 512-column half.

Perf notes: input DMAs are ordered so the K projection's operands (wk,
kvT) land first and the PE starts ~12us in; wo is fetched during
attention into recycled wk slots.  The attention loop runs ONE head at a
time (score j0/j1 -> exp -> pv j0/j1, pv emitted one chunk late): per
128-pos chunk the PE owes only 4x512-col matmuls (~1.0us) vs one
[128,1024] exp on ACT (~1.4us), so ACT paces the phase and the PE never
stalls long enough to matter.  PSUM exactly fits: 2x[128,1024] score
tiles + 2x[128,1024] pv/aux tiles = 8 banks.  All rank-1 broadcast
matmuls (q-norm, o-norm) use bf16 operands (full rate) instead of
4x-slow fp32.
"""

import sys

if "/opt/trn_rl_repo" not in sys.path:
    sys.path.insert(0, "/opt/trn_rl_repo")

import numpy as np
import ml_dtypes

import concourse.bass as bass
import concourse.bacc as bacc
import concourse.tile as tile
from concourse import mybir
from concourse.bass_utils import run_bass_kernel_spmd

BF16 = ml_dtypes.bfloat16

B, L, S, DIM = 4, 2048, 2048, 1024
H, HD = 16, 64
N_CORES = 8
LC = L // 2          # query rows per core
KC = DIM // 128      # 128-partition chunks of DIM
EPS = 1e-5
MASK_BIAS = -1.0e5   # exp(-1e5) == 0 in fp32

TRACE = False        # set by test.py for profiling
LAST_RESULT = {}     # exec_time_ns etc. for test.py

_CACHE = {}


def _build(n_sc):
    """Build the SPMD Bass program; n_sc = number of 128-wide kv chunks."""
    fp32 = mybir.dt.float32
    bf16 = mybir.dt.bfloat16
    AF = mybir.ActivationFunctionType

    nc = bacc.Bacc("TRN2", target_bir_lowering=False, debug=False,
                   num_devices=N_CORES)

    qT_d = nc.dram_tensor("qT", [DIM, LC], bf16, kind="ExternalInput")
    n_half = (n_sc + 1) // 2         # kv chunks computed locally per core
    W = n_half * 128                 # local kv width
    kvT_d = nc.dram_tensor("kvT", [DIM, W], bf16, kind="ExternalInput")
    wq_d = nc.dram_tensor("wq", [DIM, DIM], bf16, kind="ExternalInput")
    wk_d = nc.dram_tensor("wk", [DIM, DIM], bf16, kind="ExternalInput")
    wv_d = nc.dram_tensor("wv", [DIM, DIM], bf16, kind="ExternalInput")
    wo_d = nc.dram_tensor("wo", [DIM, DIM], bf16, kind="ExternalInput")
    mask01_d = nc.dram_tensor("mask01", [128, 8], fp32, kind="ExternalInput")
    qw_d = nc.dram_tensor("qw", [128, 1], fp32, kind="ExternalInput")
    kw_d = nc.dram_tensor("kw", [128, 1], fp32, kind="ExternalInput")
    ind_d = nc.dram_tensor("ind", [KC, 128, 16], bf16, kind="ExternalInput")
    ind2_d = nc.dram_tensor("ind2", [KC, 16, 128], bf16, kind="ExternalInput")
    out_d = nc.dram_tensor("out", [LC, DIM], fp32, kind="ExternalOutput")

    n_hg = (W + 511) // 512          # 512-wide groups over the local half

    with tile.TileContext(nc) as tc:
        with (
            tc.tile_pool(name="wp", bufs=3 * KC) as wp,          # wk wv wq (+wo reuse)
            tc.tile_pool(name="qtp", bufs=KC) as qtp,            # qT, later oT
            tc.tile_pool(name="kvp", bufs=KC) as kvp,            # kvT (+sq/exp tiles)
            tc.tile_pool(name="qhp", bufs=KC) as qhp,            # qhT
            tc.tile_pool(name="khp", bufs=KC) as khp,            # khT
            tc.tile_pool(name="vp", bufs=n_sc) as vp,            # vh_aug
            tc.tile_pool(name="sp", bufs=1) as sp,               # constants
            tc.tile_pool(name="skp", bufs=16) as skp,           # skT per chunk
            tc.tile_pool(name="tp", bufs=4) as tp,               # f32 temps
            tc.tile_pool(name="dp", bufs=1, space="DRAM") as dp,     # gather blobs
            tc.tile_pool(name="pa", bufs=2, space="PSUM") as pa,     # proj/score psum
            tc.tile_pool(name="po", bufs=2, space="PSUM") as po,     # pv/aux psum
        ):
            # ---------------- constants (small, land first) ----------------
            mask01_sb = sp.tile([128, 8], fp32, name="mask01")
            nc.sync.dma_start(out=mask01_sb, in_=mask01_d[:, :])
            qw_sb = sp.tile([128, 1], fp32, name="qw")
            nc.sync.dma_start(out=qw_sb, in_=qw_d[:, :])
            kw_sb = sp.tile([128, 1], fp32, name="kw")
            nc.sync.dma_start(out=kw_sb, in_=kw_d[:, :])
            ind_sb, ind2_sb = [], []
            for k in range(KC):
                i1 = sp.tile([128, 16], bf16, name=f"ind{k}")
                nc.sync.dma_start(out=i1, in_=ind_d[k, :, :])
                ind_sb.append(i1)
                i2 = sp.tile([16, 128], bf16, name=f"ind2{k}")
                nc.sync.dma_start(out=i2, in_=ind2_d[k, :, :])
                ind2_sb.append(i2)
            epsq_sb = sp.tile([16, 1], fp32, name="epsq")
            nc.vector.memset(epsq_sb, EPS)
            epsk_sb = sp.tile([16, 1], fp32, name="epsk")
            nc.vector.memset(epsk_sb, 64.0 * EPS)

            # -------- inputs: K-proj operands first, then the rest --------
            wk_sb, kvt_sb, wv_sb, wq_sb, qt_sb = [], [], [], [], []
            for k in range(KC):
                w2 = wp.tile([128, DIM], bf16, name=f"wk{k}", tag="w")
                nc.sync.dma_start(out=w2, in_=wk_d[k * 128:(k + 1) * 128, :])
                wk_sb.append(w2)
            for k in range(KC):
                kvt = kvp.tile([128, W], bf16, name=f"kvt{k}", tag="kv")
                nc.sync.dma_start(out=kvt, in_=kvT_d[k * 128:(k + 1) * 128, :])
                kvt_sb.append(kvt)
            for k in range(KC):
                w3 = wp.tile([128, DIM], bf16, name=f"wv{k}", tag="w")
                nc.sync.dma_start(out=w3, in_=wv_d[k * 128:(k + 1) * 128, :])
                wv_sb.append(w3)
            for k in range(KC):
                w1 = wp.tile([128, DIM], bf16, name=f"wq{k}", tag="w")
                nc.sync.dma_start(out=w1, in_=wq_d[k * 128:(k + 1) * 128, :])
                wq_sb.append(w1)
            for k in range(KC):
                qt = qtp.tile([128, LC], bf16, name=f"qt{k}", tag="qt")
                nc.sync.dma_start(out=qt, in_=qT_d[k * 128:(k + 1) * 128, :])
                qt_sb.append(qt)

            # -------- K projection + skT on the LOCAL kv half --------
            khh_sb = [khp.tile([128, W], bf16, name=f"khh{m}", tag="khh")
                      for m in range(KC)]
            skrec_sb = [tp.tile([16, 512], bf16, name=f"skrec{sg}",
                                tag="small16b", bufs=2)
                        for sg in range(n_hg)]
            for sg in range(n_hg):
                wdt = min(512, W - sg * 512)
                ssk = po.tile([16, 512], fp32, name="ssk", tag="pv")
                pend = None
                for m in range(KC):
                    ps = pa.tile([128, 1024], fp32, name="proj_ps", tag="pa")
                    for k in range(KC):
                        nc.tensor.matmul(
                            ps[:, :wdt],
                            lhsT=wk_sb[k][:, m * 128:(m + 1) * 128],
                            rhs=kvt_sb[k][:, sg * 512:sg * 512 + wdt],
                            start=(k == 0), stop=(k == KC - 1))
                    if pend is not None:
                        pm, pq = pend
                        nc.tensor.matmul(
                            ssk[:, :wdt], lhsT=ind_sb[pm], rhs=pq[:, :wdt],
                            start=(pm == 0), stop=False)
                    nc.vector.tensor_scalar_mul(
                        khh_sb[m][:, sg * 512:sg * 512 + wdt], ps[:, :wdt],
                        kw_sb)
                    ksq = kvp.tile([128, 1024], bf16, name="sqt", tag="sq",
                                   bufs=6)
                    nc.scalar.activation(ksq[:, :wdt], ps[:, :wdt], AF.Square)
                    pend = (m, ksq)
                pm, pq = pend
                nc.tensor.matmul(ssk[:, :wdt], lhsT=ind_sb[pm],
                                 rhs=pq[:, :wdt], start=False, stop=True)
                # 8*sqrt(mean+eps) = sqrt(sumsq + 64 eps); recip -> sk/8
                skr = tp.tile([16, 512], fp32, name="skr", tag="small16", bufs=2)
                nc.scalar.activation(skr[:, :wdt], ssk[:, :wdt], AF.Sqrt,
                                     scale=1.0, bias=epsk_sb)
                with nc.allow_low_precision(reason="k rms scale, bf16 ok"):
                    nc.vector.reciprocal(out=skrec_sb[sg][:, :wdt],
                                         in_=skr[:, :wdt])

            # -------- scale kh by sk/8, then AllGather 1 (overlaps V) ----
            VA_W = H * 65
            TOTA = KC * 128 * W
            TOTB = n_half * 128 * VA_W
            groups = [[2 * x, 2 * x + 1] for x in range(N_CORES // 2)]
            blobA_loc = dp.tile([TOTA], bf16, name="blobA_loc")
            blobA_g = dp.tile([2 * TOTA], bf16, name="blobA_g")
            for m in range(KC):
                bcsk = po.tile([128, W], fp32, name="bcsk", tag="pv")
                for sg in range(n_hg):
                    wdt = min(512, W - sg * 512)
                    nc.tensor.matmul(
                        bcsk[:, sg * 512:sg * 512 + wdt],
                        lhsT=ind2_sb[m], rhs=skrec_sb[sg][:, :wdt],
                        start=True, stop=True)
                nc.vector.tensor_mul(khh_sb[m][:, :], khh_sb[m][:, :], bcsk)
                nc.sync.dma_start(
                    out=blobA_loc[m * 128 * W:(m + 1) * 128 * W]
                    .rearrange("(p x) -> p x", p=128),
                    in_=khh_sb[m][:, :])
            nc.gpsimd.collective_compute(
                "AllGather", mybir.AluOpType.bypass, replica_groups=groups,
                ins=[blobA_loc.opt()], outs=[blobA_g.opt()])

            # -------- V projection on the LOCAL kv half (ones-augmented) ----
            va_loc = []
            for i in range(n_half):
                va = vp.tile([128, H * 65], bf16, name=f"val{i}", tag="va",
                             bufs=n_sc + 2)
                ones_cols = bass.AP(tensor=va.tensor, offset=va.offset + 64,
                                    ap=[list(va.ap[0]), [65, H], [1, 1]])
                nc.vector.memset(ones_cols, 1.0)
                ps = pa.tile([128, 1024], fp32, name="proj_ps", tag="pa")
                for k in range(KC):          # k outer: one LDW serves both jn
                    for jn in range(2):
                        nc.tensor.matmul(
                            ps[:, jn * 512:(jn + 1) * 512],
                            lhsT=kvt_sb[k][:, i * 128:(i + 1) * 128],
                            rhs=wv_sb[k][:, jn * 512:(jn + 1) * 512],
                            start=(k == 0), stop=(k == KC - 1))
                for jn in range(2):
                    dst = bass.AP(tensor=va.tensor,
                                  offset=va.offset + 65 * 8 * jn,
                                  ap=[list(va.ap[0]), [65, 8], [1, 64]])
                    nc.vector.tensor_copy(
                        dst, ps[:, jn * 512:(jn + 1) * 512]
                        .rearrange("p (h d) -> p h d", h=8))
                nc.vector.tensor_scalar_mul(va[:, :], va[:, :],
                                            mask01_sb[:, i:i + 1])
                va_loc.append(va)

            # -------- AllGather 2: va + sk (overlaps the Q projection) ----
            blobB_loc = dp.tile([TOTB], bf16, name="blobB_loc")
            blobB_g = dp.tile([2 * TOTB], bf16, name="blobB_g")
            for i in range(n_half):
                nc.sync.dma_start(
                    out=blobB_loc[i * 128 * VA_W:(i + 1) * 128 * VA_W]
                    .rearrange("(p x) -> p x", p=128),
                    in_=va_loc[i][:, :])
            nc.gpsimd.collective_compute(
                "AllGather", mybir.AluOpType.bypass, replica_groups=groups,
                ins=[blobB_loc.opt()], outs=[blobB_g.opt()])

            # ---------------- Q projection + q RMS stats ----------------
            # sumsq matmuls are emitted one (m, j) step late so the PE
            # queue head never blocks on the ACT Square.
            qh_sb = [qhp.tile([128, LC], bf16, name=f"qh{m}", tag="qh")
                     for m in range(KC)]
            sumsq_q = [po.tile([16, 512], fp32, name=f"ssq{j}", tag="pv")
                       for j in range(2)]
            pend = None                      # (m, qsq_tile)
            for m in range(KC):
                ps = pa.tile([128, 1024], fp32, name="proj_ps", tag="pa")
                for k in range(KC):          # k outer: one LDW serves both j
                    for j in range(2):
                        nc.tensor.matmul(
                            ps[:, j * 512:(j + 1) * 512],
                            lhsT=wq_sb[k][:, m * 128:(m + 1) * 128],
                            rhs=qt_sb[k][:, j * 512:(j + 1) * 512],
                            start=(k == 0), stop=(k == KC - 1))
                if pend is not None:
                    pm, pq = pend
                    for j in range(2):
                        nc.tensor.matmul(
                            sumsq_q[j][:, :], lhsT=ind_sb[pm],
                            rhs=pq[:, j * 512:(j + 1) * 512],
                            start=(pm == 0), stop=(pm == KC - 1))
                nc.vector.tensor_scalar_mul(qh_sb[m][:, :], ps[:, :], qw_sb)
                qsq = kvp.tile([128, 1024], bf16, name="sqt", tag="sq",
                               bufs=6)
                nc.scalar.activation(qsq, ps, AF.Square)
                pend = (m, qsq)
            pm, pq = pend
            for j in range(2):
                nc.tensor.matmul(sumsq_q[j][:, :], lhsT=ind_sb[pm],
                                 rhs=pq[:, j * 512:(j + 1) * 512],
                                 start=False, stop=True)
            # sq = 1/sqrt(mean + eps); broadcast with fp32r rank-1 matmuls
            sq_sb = []
            for j in range(2):
                sqr = tp.tile([16, 512], fp32, name=f"sqr{j}", tag="small16", bufs=2)
                nc.scalar.activation(sqr, sumsq_q[j][:, :], AF.Sqrt,
                                     scale=1.0 / HD, bias=epsq_sb)
                sqv = tp.tile([16, 512], bf16, name=f"sqv{j}", tag="small16b", bufs=2)
                with nc.allow_low_precision(reason="rank-1 rms scale, bf16 ok"):
                    nc.vector.reciprocal(out=sqv, in_=sqr)
                sq_sb.append(sqv)
            for m in range(KC):
                for j in range(2):
                    bc = po.tile([128, 512], fp32, name="qbc", tag="pv")
                    nc.tensor.matmul(bc, lhsT=ind2_sb[m], rhs=sq_sb[j],
                                     start=True, stop=True)
                    nc.vector.tensor_mul(
                        qh_sb[m][:, j * 512:(j + 1) * 512],
                        qh_sb[m][:, j * 512:(j + 1) * 512], bc)

            # wo fetch now: reuses wk slots (idle since K proj), lands
            # during attention.
            wo_sb = []
            for k in range(KC):
                w4 = wp.tile([128, DIM], bf16, name=f"wo{k}", tag="w")
                nc.sync.dma_start(out=w4, in_=wo_d[k * 128:(k + 1) * 128, :])
                wo_sb.append(w4)

            # -------- readback into canonical full-S tiles --------
            kh_sb = [khp.tile([128, 2 * W], bf16, name=f"kh{m}", tag="kh")
                     for m in range(KC)]
            for m in range(KC):
                for r in range(2):
                    nc.sync.dma_start(
                        out=kh_sb[m][:, r * W:(r + 1) * W],
                        in_=blobA_g[r * TOTA + m * 128 * W:
                                    r * TOTA + (m + 1) * 128 * W]
                        .rearrange("(p x) -> p x", p=128))
            va_sb = []
            for i in range(n_sc):
                r, li = i // n_half, i % n_half
                va = vp.tile([128, VA_W], bf16, name=f"va{i}", tag="va",
                             bufs=n_sc + 2)
                nc.sync.dma_start(
                    out=va[:, :],
                    in_=blobB_g[r * TOTB + li * 128 * VA_W:
                                r * TOTB + (li + 1) * 128 * VA_W]
                    .rearrange("(p x) -> p x", p=128))
                va_sb.append(va)

            # ------------- attention: one head at a time, ACT-paced -------
            # Per chunk the PE owes 4 N=512 matmuls (~1.0us) vs one
            # [128,1024] exp (~1.4us) on ACT, so ACT paces and the PE keeps
            # a 2-deep score pipeline.  pv is emitted one chunk late.  o^T
            # stays unnormalized; per-head reciprocals land in denr_sb.
            oT_sb = [qtp.tile([128, LC], bf16, name=f"oT{m}", tag="qt")
                     for m in range(KC)]
            den_sb = sp.tile([16, LC], fp32, name="den")
            nc.vector.memset(den_sb, 1.0)    # not-yet-written rows

            def emit_pv(pend):
                ph, pi, pex, ppv, first, last = pend
                for j in range(2):
                    nc.tensor.matmul(
                        ppv[:65, j * 512:(j + 1) * 512],
                        lhsT=va_sb[pi][:, ph * 65:(ph + 1) * 65],
                        rhs=pex[:, j * 512:(j + 1) * 512],
                        start=first, stop=last)
                if last:
                    m, r = ph // 2, (ph % 2) * 64
                    nc.vector.tensor_copy(oT_sb[m][r:r + 64, :], ppv[0:64, :])
                    dstage = tp.tile([128, LC], fp32, name="dstage",
                                     tag="rec", bufs=2)
                    nc.vector.tensor_copy(dstage[64:65, :], ppv[64:65, :])
                    nc.sync.dma_start(out=den_sb[ph:ph + 1, :],
                                      in_=dstage[64:65, :])

            pend = None
            for h in range(H):
                m, r = h // 2, (h % 2) * 64
                pv = po.tile([128, LC], fp32, name=f"pv{h}", tag="pv")
                for i in range(n_sc):
                    sc = pa.tile([128, LC], fp32, name="sc", tag="pa")
                    for j in range(2):
                        nc.tensor.matmul(
                            sc[:, j * 512:(j + 1) * 512],
                            lhsT=kh_sb[m][r:r + 64, i * 128:(i + 1) * 128],
                            rhs=qh_sb[m][r:r + 64, j * 512:(j + 1) * 512],
                            start=True, stop=True)
                    if pend is not None:
                        emit_pv(pend)
                    ex = kvp.tile([128, LC], bf16, name="ex", tag="sq",
                                  bufs=6)
                    nc.scalar.activation(ex, sc, AF.Exp)
                    pend = (h, i, ex, pv, i == 0, i == n_sc - 1)
            emit_pv(pend)

            # ------- normalize o^T + output projection, interleaved -------
            denr_sb = sp.tile([16, LC], bf16, name="denr")
            with nc.allow_low_precision(reason="softmax denom recip, bf16 ok"):
                nc.vector.reciprocal(out=denr_sb, in_=den_sb)
            for j in range(2):
                for m in range(KC):
                    obc = po.tile([128, 512], fp32, name="obc", tag="pv")
                    nc.tensor.matmul(obc, lhsT=ind2_sb[m],
                                     rhs=denr_sb[:, j * 512:(j + 1) * 512],
                                     start=True, stop=True)
                    nc.vector.tensor_mul(
                        oT_sb[m][:, j * 512:(j + 1) * 512],
                        oT_sb[m][:, j * 512:(j + 1) * 512], obc)
                for lc in range(4 * j, 4 * j + 4):
                    for jn in range(2):
                        ps = pa.tile([128, 1024], fp32, name="proj_ps",
                                     tag="pa")
                        for k in range(KC):
                            nc.tensor.matmul(
                                ps[:, :512],
                                lhsT=oT_sb[k][:, lc * 128:(lc + 1) * 128],
                                rhs=wo_sb[k][:, jn * 512:(jn + 1) * 512],
                                start=(k == 0), stop=(k == KC - 1))
                        osb = tp.tile([128, 512], fp32, name="osb", tag="rec",
                                      bufs=2)
                        nc.vector.tensor_copy(osb, ps[:, :512])
                        nc.sync.dma_start(
                            out=out_d[lc * 128:(lc + 1) * 128,
                                      jn * 512:(jn + 1) * 512],
                            in_=osb)
    nc.compile()
    return nc


def kernel(**inputs):
    q = np.asarray(inputs["q"], dtype=np.float32)
    kv = np.asarray(inputs["kv"], dtype=np.float32)
    seqlens = np.asarray(inputs["x_seqlens"], dtype=np.int32)
    Wq = np.asarray(inputs["Wq"], dtype=np.float32)
    Wk = np.asarray(inputs["Wk"], dtype=np.float32)
    Wv = np.asarray(inputs["Wv"], dtype=np.float32)
    Wo = np.asarray(inputs["Wo"], dtype=np.float32)
    qnw = np.asarray(inputs["q_norm_w"], dtype=np.float32)
    knw = np.asarray(inputs["k_norm_w"], dtype=np.float32)

    n_sc = max(1, int(-(-int(seqlens.max()) // 128)))
    if n_sc not in _CACHE:
        _CACHE[n_sc] = _build(n_sc)
    nc = _CACHE[n_sc]

    wq_b = np.ascontiguousarray(Wq).astype(BF16)
    wk_b = np.ascontiguousarray(Wk).astype(BF16)
    wv_b = np.ascontiguousarray(Wv).astype(BF16)
    wo_b = np.ascontiguousarray(Wo).astype(BF16)
    qw = np.tile(qnw, 2).reshape(128, 1)
    kw = np.tile(knw, 2).reshape(128, 1)
    ind = np.zeros((KC, 128, 16), np.float32)
    ind2 = np.zeros((KC, 16, 128), np.float32)
    p = np.arange(128)
    for c in range(KC):
        ind[c, p, 2 * c + p // 64] = 1.0
        ind2[c, 2 * c + p // 64, p] = 1.0
    ind = ind.astype(BF16)
    ind2 = ind2.astype(BF16)

    in_maps = []
    for c in range(N_CORES):
        b, half = c // 2, c % 2
        qT = np.ascontiguousarray(
            q[b, half * LC:(half + 1) * LC, :].T).astype(BF16)
        n_half = (n_sc + 1) // 2
        Wl = n_half * 128
        kvT = np.ascontiguousarray(
            kv[b].T[:, half * Wl:(half + 1) * Wl]).astype(BF16)
        sl = int(seqlens[b])
        gpos = half * Wl + np.arange(Wl)          # local kv global positions
        m01 = (gpos < sl).astype(np.float32).reshape(n_half, 128).T
        mask01 = np.zeros((128, 8), np.float32)
        mask01[:, :n_half] = m01
        in_maps.append({
            "qT": qT, "kvT": kvT, "wq": wq_b, "wk": wk_b, "wv": wv_b,
            "wo": wo_b, "mask01": mask01, "qw": qw, "kw": kw, "ind": ind,
            "ind2": ind2,
        })

    res = run_bass_kernel_spmd(nc, in_maps, list(range(N_CORES)),
                               trace=TRACE)
    LAST_RESULT["exec_time_ns"] = res.exec_time_ns
    LAST_RESULT["profile"] = res.profile_json

    out = np.empty((B, L, DIM), np.float32)
    for c in range(N_CORES):
        b, half = c // 2, c % 2
        out[b, half * LC:(half + 1) * LC, :] = res.results[c]["out"]
    return out


# revision 8
# speedup vs baseline: 1.5031x; 1.0780x over previous
"""Trainium2 Bass kernel for CrossAttention (B=4, L=S=2048, DIM=1024, H=16, hd=64).

Sharding: data-parallel over (batch, L-half): core c handles batch c//2,
query rows [(c%2)*1024, (c%2+1)*1024).  Each core computes the QKV
projections for its slice (K/V duplicated within a batch pair), per-head
RMSNorm, masked softmax attention, and the output projection.

Device layout is feature-major ("transposed"): activations live as
[dim, tokens] so every matmul contraction dim is on SBUF partitions with
no on-device transposes.  The host pre-transposes q/kv and casts to bf16.

Softmax: after RMS norm |score| <= 8, so no running max is needed.  The
k-norm rsqrt/8 is folded into kh right after the K projection (rank-2 PE
broadcast via the head-indicator matmul), and the padding mask is folded
into V by zeroing masked va rows (masked exps are finite but contribute
nothing, including to the denominator's ones-column) -- the
attention-phase exp is therefore a BARE activation with no scale/bias AP
reads (~850ns vs ~1400ns per [128,1024] tile).  The denominator comes
from a 65th "ones" column appended to V; the division is deferred: o^T
is stored unnormalized, den rows are collected per head, one fast
approximate reciprocal runs at the end, and a rank-1 bf16 PE broadcast
normalizes o^T interleaved with the output projection per 512-col half.

Perf notes: the Sync engine costs ~650ns per dma_start trigger, so every
multi-tile transfer is batched into ONE dma_start over a big tile (wk,
kvt, wv, wq, qt, wo, ind, ind2 inputs; kh/va gather blobs in
partition-major layout; readbacks).  K-proj operands (wk, kvT) are
issued first so the PE starts ~13us in; wo is fetched during attention
into the recycled wk slot.  The attention loop runs ONE head at a time;
pv matmuls are emitted TWO chunks late so they never wait on the exp:
per 128-pos chunk the PE owes 4 N=512 matmuls (~0.95us) vs one
[128,1024] exp on ACT (~1.1us), so ACT paces the phase.  PSUM exactly
fits: 2x[128,1024] score tiles + 2x[128,1024] pv/aux tiles = 8 banks.
Rank-1 broadcast matmuls (q-norm, k-norm, o-norm) use bf16 operands
(full rate) instead of 4x-slow fp32.
"""

import sys

if "/opt/trn_rl_repo" not in sys.path:
    sys.path.insert(0, "/opt/trn_rl_repo")

from collections import deque

import numpy as np
import ml_dtypes

import concourse.bass as bass
import concourse.bacc as bacc
import concourse.tile as tile
from concourse import mybir
from concourse.bass_utils import run_bass_kernel_spmd

BF16 = ml_dtypes.bfloat16

B, L, S, DIM = 4, 2048, 2048, 1024
H, HD = 16, 64
N_CORES = 8
LC = L // 2          # query rows per core
KC = DIM // 128      # 128-partition chunks of DIM
EPS = 1e-5

TRACE = False        # set by test.py for profiling
LAST_RESULT = {}     # exec_time_ns etc. for test.py

_CACHE = {}


def _build(n_sc):
    """Build the SPMD Bass program; n_sc = number of 128-wide kv chunks."""
    fp32 = mybir.dt.float32
    bf16 = mybir.dt.bfloat16
    AF = mybir.ActivationFunctionType

    nc = bacc.Bacc("TRN2", target_bir_lowering=False, debug=False,
                   num_devices=N_CORES)

    qT_d = nc.dram_tensor("qT", [DIM, LC], bf16, kind="ExternalInput")
    n_half = (n_sc + 1) // 2         # kv chunks computed locally per core
    W = n_half * 128                 # local kv width
    kvT_d = nc.dram_tensor("kvT", [DIM, W], bf16, kind="ExternalInput")
    wq_d = nc.dram_tensor("wq", [DIM, DIM], bf16, kind="ExternalInput")
    wk_d = nc.dram_tensor("wk", [DIM, DIM], bf16, kind="ExternalInput")
    wv_d = nc.dram_tensor("wv", [DIM, DIM], bf16, kind="ExternalInput")
    wo_d = nc.dram_tensor("wo", [DIM, DIM], bf16, kind="ExternalInput")
    mask01_d = nc.dram_tensor("mask01", [128, 8], fp32, kind="ExternalInput")
    qw_d = nc.dram_tensor("qw", [128, 1], fp32, kind="ExternalInput")
    kw_d = nc.dram_tensor("kw", [128, 1], fp32, kind="ExternalInput")
    ind_d = nc.dram_tensor("ind", [KC, 128, 16], bf16, kind="ExternalInput")
    ind2_d = nc.dram_tensor("ind2", [KC, 16, 128], bf16, kind="ExternalInput")
    out_d = nc.dram_tensor("out", [LC, DIM], fp32, kind="ExternalOutput")

    n_hg = (W + 511) // 512          # 512-wide groups over the local half
    VA_W = H * 65                    # 1040 va columns per chunk

    with tile.TileContext(nc) as tc:
        with (
            tc.tile_pool(name="wp", bufs=3) as wp,               # wk wv wq (+wo)
            tc.tile_pool(name="qtp", bufs=1) as qtp,             # qT, later oT
            tc.tile_pool(name="kvp", bufs=1) as kvp,             # kvT (+sq/exp)
            tc.tile_pool(name="qhp", bufs=KC) as qhp,            # qhT
            tc.tile_pool(name="khp", bufs=1) as khp,             # khT
            tc.tile_pool(name="vp", bufs=1) as vp,               # vh_aug
            tc.tile_pool(name="sp", bufs=1) as sp,               # constants
            tc.tile_pool(name="tp", bufs=4) as tp,               # f32 temps
            tc.tile_pool(name="dp", bufs=1, space="DRAM") as dp,     # blobs
            tc.tile_pool(name="pa", bufs=2, space="PSUM") as pa,     # proj/score
            tc.tile_pool(name="po", bufs=2, space="PSUM") as po,     # pv/aux
        ):
            # ---- inputs, one dma_start each; K-proj operands first ----
            wk_big = wp.tile([128, KC * DIM], bf16, name="wk", tag="wbig")
            nc.sync.dma_start(
                out=wk_big.rearrange("p (k j) -> p k j", k=KC),
                in_=wk_d.rearrange("(k p) j -> p k j", k=KC))
            kvt_big = kvp.tile([128, KC * W], bf16, name="kvt", tag="kv")
            nc.sync.dma_start(
                out=kvt_big.rearrange("p (k x) -> p k x", k=KC),
                in_=kvT_d.rearrange("(k p) x -> p k x", k=KC))
            ind_big = sp.tile([128, KC * 16], bf16, name="ind")
            nc.sync.dma_start(
                out=ind_big.rearrange("p (k x) -> p k x", k=KC),
                in_=ind_d.rearrange("k p x -> p k x"))
            kw_sb = sp.tile([128, 1], fp32, name="kw")
            nc.sync.dma_start(out=kw_sb, in_=kw_d[:, :])
            ind2_big = sp.tile([16, KC * 128], bf16, name="ind2")
            nc.sync.dma_start(
                out=ind2_big.rearrange("p (k x) -> p k x", k=KC),
                in_=ind2_d.rearrange("k p x -> p k x"))
            mask01_sb = sp.tile([128, 8], fp32, name="mask01")
            nc.sync.dma_start(out=mask01_sb, in_=mask01_d[:, :])
            qw_sb = sp.tile([128, 1], fp32, name="qw")
            nc.sync.dma_start(out=qw_sb, in_=qw_d[:, :])
            wv_big = wp.tile([128, KC * DIM], bf16, name="wv", tag="wbig")
            nc.sync.dma_start(
                out=wv_big.rearrange("p (k j) -> p k j", k=KC),
                in_=wv_d.rearrange("(k p) j -> p k j", k=KC))
            wq_big = wp.tile([128, KC * DIM], bf16, name="wq", tag="wbig")
            nc.sync.dma_start(
                out=wq_big.rearrange("p (k j) -> p k j", k=KC),
                in_=wq_d.rearrange("(k p) j -> p k j", k=KC))
            qt_big = qtp.tile([128, KC * LC], bf16, name="qt", tag="qt")
            nc.sync.dma_start(
                out=qt_big.rearrange("p (k j) -> p k j", k=KC),
                in_=qT_d.rearrange("(k p) j -> p k j", k=KC))

            def wkc(k):
                return wk_big[:, k * DIM:(k + 1) * DIM]

            def wvc(k):
                return wv_big[:, k * DIM:(k + 1) * DIM]

            def wqc(k):
                return wq_big[:, k * DIM:(k + 1) * DIM]

            def qtc(k):
                return qt_big[:, k * LC:(k + 1) * LC]

            def kvtc(k):
                return kvt_big[:, k * W:(k + 1) * W]

            def indc(k):
                return ind_big[:, k * 16:(k + 1) * 16]

            def ind2c(k):
                return ind2_big[:, k * 128:(k + 1) * 128]

            epsq_sb = sp.tile([16, 1], fp32, name="epsq")
            nc.vector.memset(epsq_sb, EPS)
            epsk_sb = sp.tile([16, 1], fp32, name="epsk")
            nc.vector.memset(epsk_sb, 64.0 * EPS)

            # -------- K projection + skT on the LOCAL kv half --------
            khh_big = khp.tile([128, KC * W], bf16, name="khh", tag="khh")
            skrec_sb = [tp.tile([16, 512], bf16, name=f"skrec{sg}",
                                tag="small16b", bufs=2)
                        for sg in range(n_hg)]
            for sg in range(n_hg):
                wdt = min(512, W - sg * 512)
                ssk = po.tile([16, 512], fp32, name="ssk", tag="pv")
                pend = None
                for m in range(KC):
                    ps = pa.tile([128, 1024], fp32, name="proj_ps", tag="pa")
                    for k in range(KC):
                        nc.tensor.matmul(
                            ps[:, :wdt],
                            lhsT=wkc(k)[:, m * 128:(m + 1) * 128],
                            rhs=kvtc(k)[:, sg * 512:sg * 512 + wdt],
                            start=(k == 0), stop=(k == KC - 1))
                    if pend is not None:
                        pm, pq = pend
                        nc.tensor.matmul(
                            ssk[:, :wdt], lhsT=indc(pm), rhs=pq[:, :wdt],
                            start=(pm == 0), stop=False)
                    nc.vector.tensor_scalar_mul(
                        khh_big[:, m * W + sg * 512:m * W + sg * 512 + wdt],
                        ps[:, :wdt], kw_sb)
                    ksq = kvp.tile([128, 1024], bf16, name="sqt", tag="sq",
                                   bufs=6)
                    nc.scalar.activation(ksq[:, :wdt], ps[:, :wdt], AF.Square)
                    pend = (m, ksq)
                pm, pq = pend
                nc.tensor.matmul(ssk[:, :wdt], lhsT=indc(pm),
                                 rhs=pq[:, :wdt], start=False, stop=True)
                # 8*sqrt(mean+eps) = sqrt(sumsq + 64 eps); recip -> sk/8
                skr = tp.tile([16, 512], fp32, name="skr", tag="small16",
                              bufs=2)
                nc.scalar.activation(skr[:, :wdt], ssk[:, :wdt], AF.Sqrt,
                                     scale=1.0, bias=epsk_sb)
                with nc.allow_low_precision(reason="k rms scale, bf16 ok"):
                    nc.vector.reciprocal(out=skrec_sb[sg][:, :wdt],
                                         in_=skr[:, :wdt])

            # -------- scale kh by sk/8, then AllGather 1 (overlaps V) ----
            TOTA = KC * 128 * W
            TOTB = n_half * 128 * VA_W
            groups = [[2 * x, 2 * x + 1] for x in range(N_CORES // 2)]
            blobA_loc = dp.tile([TOTA], bf16, name="blobA_loc")
            blobA_g = dp.tile([2 * TOTA], bf16, name="blobA_g")
            for m in range(KC):
                bcsk = po.tile([128, W], fp32, name="bcsk", tag="pv")
                for sg in range(n_hg):
                    wdt = min(512, W - sg * 512)
                    nc.tensor.matmul(
                        bcsk[:, sg * 512:sg * 512 + wdt],
                        lhsT=ind2c(m), rhs=skrec_sb[sg][:, :wdt],
                        start=True, stop=True)
                nc.vector.tensor_mul(khh_big[:, m * W:(m + 1) * W],
                                     khh_big[:, m * W:(m + 1) * W], bcsk)
            # p-major blob: (p, m, x)
            nc.sync.dma_start(
                out=blobA_loc.rearrange("(p x) -> p x", p=128),
                in_=khh_big[:, :])
            nc.gpsimd.collective_compute(
                "AllGather", mybir.AluOpType.bypass, replica_groups=groups,
                ins=[blobA_loc.opt()], outs=[blobA_g.opt()])

            # -------- V projection on the LOCAL kv half (ones-augmented) ----
            val_big = vp.tile([128, n_half * VA_W], bf16, name="val",
                              tag="val")
            for i in range(n_half):
                va = val_big[:, i * VA_W:(i + 1) * VA_W]
                ones_cols = bass.AP(tensor=va.tensor, offset=va.offset + 64,
                                    ap=[list(va.ap[0]), [65, H], [1, 1]])
                nc.vector.memset(ones_cols, 1.0)
                ps = pa.tile([128, 1024], fp32, name="proj_ps", tag="pa")
                for k in range(KC):          # k outer: one LDW serves both jn
                    for jn in range(2):
                        nc.tensor.matmul(
                            ps[:, jn * 512:(jn + 1) * 512],
                            lhsT=kvtc(k)[:, i * 128:(i + 1) * 128],
                            rhs=wvc(k)[:, jn * 512:(jn + 1) * 512],
                            start=(k == 0), stop=(k == KC - 1))
                for jn in range(2):
                    dst = bass.AP(tensor=va.tensor,
                                  offset=va.offset + 65 * 8 * jn,
                                  ap=[list(va.ap[0]), [65, 8], [1, 64]])
                    nc.vector.tensor_copy(
                        dst, ps[:, jn * 512:(jn + 1) * 512]
                        .rearrange("p (h d) -> p h d", h=8))
                nc.vector.tensor_scalar_mul(va, va, mask01_sb[:, i:i + 1])

            # -------- AllGather 2: va (overlaps the Q projection) ----
            blobB_loc = dp.tile([TOTB], bf16, name="blobB_loc")
            blobB_g = dp.tile([2 * TOTB], bf16, name="blobB_g")
            nc.sync.dma_start(
                out=blobB_loc.rearrange("(p x) -> p x", p=128),
                in_=val_big[:, :])
            nc.gpsimd.collective_compute(
                "AllGather", mybir.AluOpType.bypass, replica_groups=groups,
                ins=[blobB_loc.opt()], outs=[blobB_g.opt()])

            # ---------------- Q projection + q RMS stats ----------------
            # sumsq matmuls are emitted one (m, j) step late so the PE
            # queue head never blocks on the ACT Square.
            qh_sb = [qhp.tile([128, LC], bf16, name=f"qh{m}", tag="qh")
                     for m in range(KC)]
            sumsq_q = [po.tile([16, 512], fp32, name=f"ssq{j}", tag="pv")
                       for j in range(2)]
            pend = None                      # (m, qsq_tile)
            for m in range(KC):
                ps = pa.tile([128, 1024], fp32, name="proj_ps", tag="pa")
                for k in range(KC):          # k outer: one LDW serves both j
                    for j in range(2):
                        nc.tensor.matmul(
                            ps[:, j * 512:(j + 1) * 512],
                            lhsT=wqc(k)[:, m * 128:(m + 1) * 128],
                            rhs=qtc(k)[:, j * 512:(j + 1) * 512],
                            start=(k == 0), stop=(k == KC - 1))
                if pend is not None:
                    pm, pq = pend
                    for j in range(2):
                        nc.tensor.matmul(
                            sumsq_q[j][:, :], lhsT=indc(pm),
                            rhs=pq[:, j * 512:(j + 1) * 512],
                            start=(pm == 0), stop=(pm == KC - 1))
                nc.vector.tensor_scalar_mul(qh_sb[m][:, :], ps[:, :], qw_sb)
                qsq = kvp.tile([128, 1024], bf16, name="sqt", tag="sq",
                               bufs=6)
                nc.scalar.activation(qsq, ps, AF.Square)
                pend = (m, qsq)
            pm, pq = pend
            for j in range(2):
                nc.tensor.matmul(sumsq_q[j][:, :], lhsT=indc(pm),
                                 rhs=pq[:, j * 512:(j + 1) * 512],
                                 start=False, stop=True)
            # sq = 1/sqrt(mean + eps); broadcast with bf16 rank-1 matmuls
            sq_sb = []
            for j in range(2):
                sqr = tp.tile([16, 512], fp32, name=f"sqr{j}", tag="small16",
                              bufs=2)
                nc.scalar.activation(sqr, sumsq_q[j][:, :], AF.Sqrt,
                                     scale=1.0 / HD, bias=epsq_sb)
                sqv = tp.tile([16, 512], bf16, name=f"sqv{j}", tag="small16b",
                              bufs=2)
                with nc.allow_low_precision(reason="rank-1 rms scale, bf16"):
                    nc.vector.reciprocal(out=sqv, in_=sqr)
                sq_sb.append(sqv)
            for m in range(KC):
                for j in range(2):
                    bc = po.tile([128, 512], fp32, name="qbc", tag="pv")
                    nc.tensor.matmul(bc, lhsT=ind2c(m), rhs=sq_sb[j],
                                     start=True, stop=True)
                    nc.vector.tensor_mul(
                        qh_sb[m][:, j * 512:(j + 1) * 512],
                        qh_sb[m][:, j * 512:(j + 1) * 512], bc)

            # -------- readback into canonical full-S tiles --------
            kh_big = khp.tile([128, KC * 2 * W], bf16, name="kh", tag="kh")
            for r in range(2):
                nc.sync.dma_start(
                    out=kh_big.rearrange("p (m rx) -> p m rx", m=KC)
                    [:, :, r * W:(r + 1) * W],
                    in_=blobA_g[r * TOTA:(r + 1) * TOTA]
                    .rearrange("(p m x) -> p m x", p=128, m=KC))
            va_big = vp.tile([128, 2 * n_half * VA_W], bf16, name="vab",
                             tag="vab")
            nc.sync.dma_start(
                out=va_big.rearrange("p (r y) -> p r y", r=2),
                in_=blobB_g.rearrange("(r p y) -> p r y", r=2, p=128))

            def khc(m):
                return kh_big[:, m * 2 * W:(m + 1) * 2 * W]

            def vac(i):
                return va_big[:, i * VA_W:(i + 1) * VA_W]

            # wo fetch now: reuses the wk slot (idle since K proj), lands
            # during attention.
            wo_big = wp.tile([128, KC * DIM], bf16, name="wo", tag="wbig")
            nc.sync.dma_start(
                out=wo_big.rearrange("p (k j) -> p k j", k=KC),
                in_=wo_d.rearrange("(k p) j -> p k j", k=KC))

            def woc(k):
                return wo_big[:, k * DIM:(k + 1) * DIM]

            # ------------- attention: one head at a time, ACT-paced -------
            # pv emitted TWO chunks late so it never waits on the exp; the
            # PE keeps a 2-deep score pipeline in the other direction.
            oT_big = qtp.tile([128, KC * LC], bf16, name="oT", tag="qt")

            def oTc(m):
                return oT_big[:, m * LC:(m + 1) * LC]

            den_sb = sp.tile([16, LC], fp32, name="den")
            nc.vector.memset(den_sb, 1.0)    # not-yet-written rows

            def emit_pv(pe):
                ph, pi, pex, ppv, first, last = pe
                for j in range(2):
                    nc.tensor.matmul(
                        ppv[:65, j * 512:(j + 1) * 512],
                        lhsT=vac(pi)[:, ph * 65:(ph + 1) * 65],
                        rhs=pex[:, j * 512:(j + 1) * 512],
                        start=first, stop=last)
                if last:
                    m, r = ph // 2, (ph % 2) * 64
                    dstage = tp.tile([128, LC], fp32, name="dstage",
                                     tag="rec", bufs=2)
                    nc.vector.tensor_copy(dstage[64:65, :], ppv[64:65, :])
                    nc.sync.dma_start(out=den_sb[ph:ph + 1, :],
                                      in_=dstage[64:65, :])
                    nc.vector.tensor_copy(oTc(m)[r:r + 64, :], ppv[0:64, :])

            pending = deque()
            for h in range(H):
                m, r = h // 2, (h % 2) * 64
                pv = po.tile([128, LC], fp32, name=f"pv{h}", tag="pv")
                for i in range(n_sc):
                    sc = pa.tile([128, LC], fp32, name="sc", tag="pa")
                    for j in range(2):
                        nc.tensor.matmul(
                            sc[:, j * 512:(j + 1) * 512],
                            lhsT=khc(m)[r:r + 64, i * 128:(i + 1) * 128],
                            rhs=qh_sb[m][r:r + 64, j * 512:(j + 1) * 512],
                            start=True, stop=True)
                    if len(pending) == 2:
                        emit_pv(pending.popleft())
                    ex = kvp.tile([128, LC], bf16, name="ex", tag="sq",
                                  bufs=6)
                    nc.scalar.activation(ex, sc, AF.Exp)
                    pending.append((h, i, ex, pv, i == 0, i == n_sc - 1))
            while pending:
                emit_pv(pending.popleft())

            # ------- normalize o^T + output projection, interleaved -------
            denr32 = tp.tile([16, LC], fp32, name="denr32", tag="rec",
                             bufs=2)
            nc.vector.reciprocal_approx_fast(out=denr32, in_=den_sb)
            denr_sb = sp.tile([16, LC], bf16, name="denr")
            nc.vector.tensor_copy(denr_sb, denr32)
            for j in range(2):
                for m in range(KC):
                    obc = po.tile([128, 512], fp32, name="obc", tag="pv")
                    nc.tensor.matmul(obc, lhsT=ind2c(m),
                                     rhs=denr_sb[:, j * 512:(j + 1) * 512],
                                     start=True, stop=True)
                    nc.vector.tensor_mul(
                        oTc(m)[:, j * 512:(j + 1) * 512],
                        oTc(m)[:, j * 512:(j + 1) * 512], obc)
                for lc in range(4 * j, 4 * j + 4):
                    for jn in range(2):
                        ps = pa.tile([128, 1024], fp32, name="proj_ps",
                                     tag="pa")
                        for k in range(KC):
                            nc.tensor.matmul(
                                ps[:, :512],
                                lhsT=oTc(k)[:, lc * 128:(lc + 1) * 128],
                                rhs=woc(k)[:, jn * 512:(jn + 1) * 512],
                                start=(k == 0), stop=(k == KC - 1))
                        osb = tp.tile([128, 512], fp32, name="osb", tag="rec",
                                      bufs=2)
                        nc.vector.tensor_copy(osb, ps[:, :512])
                        nc.sync.dma_start(
                            out=out_d[lc * 128:(lc + 1) * 128,
                                      jn * 512:(jn + 1) * 512],
                            in_=osb)
    nc.compile()
    return nc


def kernel(**inputs):
    q = np.asarray(inputs["q"], dtype=np.float32)
    kv = np.asarray(inputs["kv"], dtype=np.float32)
    seqlens = np.asarray(inputs["x_seqlens"], dtype=np.int32)
    Wq = np.asarray(inputs["Wq"], dtype=np.float32)
    Wk = np.asarray(inputs["Wk"], dtype=np.float32)
    Wv = np.asarray(inputs["Wv"], dtype=np.float32)
    Wo = np.asarray(inputs["Wo"], dtype=np.float32)
    qnw = np.asarray(inputs["q_norm_w"], dtype=np.float32)
    knw = np.asarray(inputs["k_norm_w"], dtype=np.float32)

    n_sc = max(1, int(-(-int(seqlens.max()) // 128)))
    if n_sc not in _CACHE:
        _CACHE[n_sc] = _build(n_sc)
    nc = _CACHE[n_sc]

    wq_b = np.ascontiguousarray(Wq).astype(BF16)
    wk_b = np.ascontiguousarray(Wk).astype(BF16)
    wv_b = np.ascontiguousarray(Wv).astype(BF16)
    wo_b = np.ascontiguousarray(Wo).astype(BF16)
    qw = np.tile(qnw, 2).reshape(128, 1)
    kw = np.tile(knw, 2).reshape(128, 1)
    ind = np.zeros((KC, 128, 16), np.float32)
    ind2 = np.zeros((KC, 16, 128), np.float32)
    p = np.arange(128)
    for c in range(KC):
        ind[c, p, 2 * c + p // 64] = 1.0
        ind2[c, 2 * c + p // 64, p] = 1.0
    ind = ind.astype(BF16)
    ind2 = ind2.astype(BF16)

    in_maps = []
    for c in range(N_CORES):
        b, half = c // 2, c % 2
        qT = np.ascontiguousarray(
            q[b, half * LC:(half + 1) * LC, :].T).astype(BF16)
        n_half = (n_sc + 1) // 2
        Wl = n_half * 128
        kvT = np.ascontiguousarray(
            kv[b].T[:, half * Wl:(half + 1) * Wl]).astype(BF16)
        sl = int(seqlens[b])
        gpos = half * Wl + np.arange(Wl)          # local kv global positions
        m01 = (gpos < sl).astype(np.float32).reshape(n_half, 128).T
        mask01 = np.zeros((128, 8), np.float32)
        mask01[:, :n_half] = m01
        in_maps.append({
            "qT": qT, "kvT": kvT, "wq": wq_b, "wk": wk_b, "wv": wv_b,
            "wo": wo_b, "mask01": mask01, "qw": qw, "kw": kw, "ind": ind,
            "ind2": ind2,
        })

    res = run_bass_kernel_spmd(nc, in_maps, list(range(N_CORES)),
                               trace=TRACE)
    LAST_RESULT["exec_time_ns"] = res.exec_time_ns
    LAST_RESULT["profile"] = res.profile_json

    out = np.empty((B, L, DIM), np.float32)
    for c in range(N_CORES):
        b, half = c // 2, c % 2
        out[b, half * LC:(half + 1) * LC, :] = res.results[c]["out"]
    return out


# revision 9
# speedup vs baseline: 1.5271x; 1.0160x over previous
"""Trainium2 Bass kernel for CrossAttention (B=4, L=S=2048, DIM=1024, H=16, hd=64).

Sharding: data-parallel over (batch, L-half): core c handles batch c//2,
query rows [(c%2)*1024, (c%2+1)*1024).  Each core computes the QKV
projections for its slice (K/V duplicated within a batch pair), per-head
RMSNorm, masked softmax attention, and the output projection.

Device layout is feature-major ("transposed"): activations live as
[dim, tokens] so every matmul contraction dim is on SBUF partitions with
no on-device transposes.  The host pre-transposes q/kv and casts to bf16.

Softmax: after RMS norm |score| <= 8, so no running max is needed.  The
k-norm rsqrt/8 is folded into kh right after the K projection (rank-2 PE
broadcast via the head-indicator matmul), and the padding mask is folded
into V by zeroing masked va rows (masked exps are finite but contribute
nothing, including to the denominator's ones-column) -- the
attention-phase exp is therefore a BARE activation with no scale/bias AP
reads (~850ns vs ~1400ns per [128,1024] tile).  The denominator comes
from a 65th "ones" column appended to V; the division is deferred: o^T
is stored unnormalized, den rows are collected per head, one fast
approximate reciprocal runs at the end, and a rank-1 bf16 PE broadcast
normalizes o^T interleaved with the output projection per 512-col half.

Perf notes: the Sync engine costs ~650ns per dma_start trigger, so every
multi-tile transfer is batched into ONE dma_start over a big tile (wk,
kvt, wv, wq, qt, wo, ind, ind2 inputs; kh/va gather blobs in
partition-major layout; readbacks).  K-proj operands (wk, kvT) are
issued first so the PE starts ~13us in; wo is fetched during attention
into the recycled wk slot.  The attention loop runs ONE head at a time;
pv matmuls are emitted TWO chunks late so they never wait on the exp:
per 128-pos chunk the PE owes 4 N=512 matmuls (~0.95us) vs one
[128,1024] exp on ACT (~1.1us), so ACT paces the phase.  PSUM exactly
fits: 2x[128,1024] score tiles + 2x[128,1024] pv/aux tiles = 8 banks.
Rank-1 broadcast matmuls (q-norm, k-norm, o-norm) use bf16 operands
(full rate) instead of 4x-slow fp32.
"""

import sys

if "/opt/trn_rl_repo" not in sys.path:
    sys.path.insert(0, "/opt/trn_rl_repo")

from collections import deque

import numpy as np
import ml_dtypes

import concourse.bass as bass
import concourse.bacc as bacc
import concourse.tile as tile
from concourse import mybir
from concourse.bass_utils import run_bass_kernel_spmd

BF16 = ml_dtypes.bfloat16

B, L, S, DIM = 4, 2048, 2048, 1024
H, HD = 16, 64
N_CORES = 8
LC = L // 2          # query rows per core
KC = DIM // 128      # 128-partition chunks of DIM
EPS = 1e-5

TRACE = False        # set by test.py for profiling
LAST_RESULT = {}     # exec_time_ns etc. for test.py

_CACHE = {}


def _build(n_sc):
    """Build the SPMD Bass program; n_sc = number of 128-wide kv chunks."""
    fp32 = mybir.dt.float32
    bf16 = mybir.dt.bfloat16
    AF = mybir.ActivationFunctionType

    nc = bacc.Bacc("TRN2", target_bir_lowering=False, debug=False,
                   num_devices=N_CORES)

    qT_d = nc.dram_tensor("qT", [128, KC * LC], bf16, kind="ExternalInput")
    n_half = (n_sc + 1) // 2         # kv chunks computed locally per core
    W = n_half * 128                 # local kv width
    kvT_d = nc.dram_tensor("kvT", [128, KC * W], bf16, kind="ExternalInput")
    wq_d = nc.dram_tensor("wq", [128, KC * DIM], bf16, kind="ExternalInput")
    wk_d = nc.dram_tensor("wk", [128, KC * DIM], bf16, kind="ExternalInput")
    wv_d = nc.dram_tensor("wv", [128, KC * DIM], bf16, kind="ExternalInput")
    wo_d = nc.dram_tensor("wo", [128, KC * DIM], bf16, kind="ExternalInput")
    mask01_d = nc.dram_tensor("mask01", [128, 8], fp32, kind="ExternalInput")
    qw_d = nc.dram_tensor("qw", [128, 1], fp32, kind="ExternalInput")
    kw_d = nc.dram_tensor("kw", [128, 1], fp32, kind="ExternalInput")
    ind_d = nc.dram_tensor("ind", [128, KC * 16], bf16, kind="ExternalInput")
    ind2_d = nc.dram_tensor("ind2", [16, KC * 128], bf16,
                            kind="ExternalInput")
    out_d = nc.dram_tensor("out", [LC, DIM], fp32, kind="ExternalOutput")

    n_hg = (W + 511) // 512          # 512-wide groups over the local half
    VA_W = H * 65                    # 1040 va columns per chunk

    with tile.TileContext(nc) as tc:
        with (
            tc.tile_pool(name="wp", bufs=3) as wp,               # wk wv wq (+wo)
            tc.tile_pool(name="qtp", bufs=1) as qtp,             # qT, later oT
            tc.tile_pool(name="kvp", bufs=1) as kvp,             # kvT (+sq/exp)
            tc.tile_pool(name="qhp", bufs=KC) as qhp,            # qhT
            tc.tile_pool(name="khp", bufs=1) as khp,             # khT
            tc.tile_pool(name="vp", bufs=1) as vp,               # vh_aug
            tc.tile_pool(name="sp", bufs=1) as sp,               # constants
            tc.tile_pool(name="tp", bufs=4) as tp,               # f32 temps
            tc.tile_pool(name="dp", bufs=1, space="DRAM") as dp,     # blobs
            tc.tile_pool(name="pa", bufs=2, space="PSUM") as pa,     # proj/score
            tc.tile_pool(name="po", bufs=2, space="PSUM") as po,     # pv/aux
        ):
            # ---- inputs, one dma_start each; K-proj operands first ----
            wk_big = wp.tile([128, KC * DIM], bf16, name="wk", tag="wbig")
            nc.sync.dma_start(out=wk_big, in_=wk_d[:, :])
            kvt_big = kvp.tile([128, KC * W], bf16, name="kvt", tag="kv")
            nc.sync.dma_start(out=kvt_big, in_=kvT_d[:, :])
            ind_big = sp.tile([128, KC * 16], bf16, name="ind")
            nc.sync.dma_start(out=ind_big, in_=ind_d[:, :])
            kw_sb = sp.tile([128, 1], fp32, name="kw")
            nc.sync.dma_start(out=kw_sb, in_=kw_d[:, :])
            ind2_big = sp.tile([16, KC * 128], bf16, name="ind2")
            nc.sync.dma_start(out=ind2_big, in_=ind2_d[:, :])
            mask01_sb = sp.tile([128, 8], fp32, name="mask01")
            nc.sync.dma_start(out=mask01_sb, in_=mask01_d[:, :])
            qw_sb = sp.tile([128, 1], fp32, name="qw")
            nc.sync.dma_start(out=qw_sb, in_=qw_d[:, :])
            wv_big = wp.tile([128, KC * DIM], bf16, name="wv", tag="wbig")
            nc.sync.dma_start(out=wv_big, in_=wv_d[:, :])
            wq_big = wp.tile([128, KC * DIM], bf16, name="wq", tag="wbig")
            nc.sync.dma_start(out=wq_big, in_=wq_d[:, :])
            qt_big = qtp.tile([128, KC * LC], bf16, name="qt", tag="qt")
            nc.sync.dma_start(out=qt_big, in_=qT_d[:, :])
            # warmup: pay the one-time CC collective-launch cost (~12us)
            # during the input DMA phase with a tiny dependency-free gather
            wu_in = dp.tile([128], bf16, name="wu_in")
            wu_out = dp.tile([256], bf16, name="wu_out")
            nc.gpsimd.collective_compute(
                "AllGather", mybir.AluOpType.bypass,
                replica_groups=[[2 * x, 2 * x + 1]
                                for x in range(N_CORES // 2)],
                ins=[wu_in.opt()], outs=[wu_out.opt()])

            def wkc(k):
                return wk_big[:, k * DIM:(k + 1) * DIM]

            def wvc(k):
                return wv_big[:, k * DIM:(k + 1) * DIM]

            def wqc(k):
                return wq_big[:, k * DIM:(k + 1) * DIM]

            def qtc(k):
                return qt_big[:, k * LC:(k + 1) * LC]

            def kvtc(k):
                return kvt_big[:, k * W:(k + 1) * W]

            def indc(k):
                return ind_big[:, k * 16:(k + 1) * 16]

            def ind2c(k):
                return ind2_big[:, k * 128:(k + 1) * 128]

            epsq_sb = sp.tile([16, 1], fp32, name="epsq")
            nc.vector.memset(epsq_sb, EPS)
            epsk_sb = sp.tile([16, 1], fp32, name="epsk")
            nc.vector.memset(epsk_sb, 64.0 * EPS)

            # -------- K projection + skT on the LOCAL kv half --------
            khh_big = khp.tile([128, KC * W], bf16, name="khh", tag="khh")
            skrec_sb = [tp.tile([16, 512], bf16, name=f"skrec{sg}",
                                tag="small16b", bufs=2)
                        for sg in range(n_hg)]
            for sg in range(n_hg):
                wdt = min(512, W - sg * 512)
                ssk = po.tile([16, 512], fp32, name="ssk", tag="pv")
                pend = None
                for m in range(KC):
                    ps = pa.tile([128, 1024], fp32, name="proj_ps", tag="pa")
                    for k in range(KC):
                        nc.tensor.matmul(
                            ps[:, :wdt],
                            lhsT=wkc(k)[:, m * 128:(m + 1) * 128],
                            rhs=kvtc(k)[:, sg * 512:sg * 512 + wdt],
                            start=(k == 0), stop=(k == KC - 1))
                    if pend is not None:
                        pm, pq = pend
                        nc.tensor.matmul(
                            ssk[:, :wdt], lhsT=indc(pm), rhs=pq[:, :wdt],
                            start=(pm == 0), stop=False)
                    nc.vector.tensor_scalar_mul(
                        khh_big[:, m * W + sg * 512:m * W + sg * 512 + wdt],
                        ps[:, :wdt], kw_sb)
                    ksq = kvp.tile([128, 1024], bf16, name="sqt", tag="sq",
                                   bufs=6)
                    nc.scalar.activation(ksq[:, :wdt], ps[:, :wdt], AF.Square)
                    pend = (m, ksq)
                pm, pq = pend
                nc.tensor.matmul(ssk[:, :wdt], lhsT=indc(pm),
                                 rhs=pq[:, :wdt], start=False, stop=True)
                # 8*sqrt(mean+eps) = sqrt(sumsq + 64 eps); recip -> sk/8
                skr = tp.tile([16, 512], fp32, name="skr", tag="small16",
                              bufs=2)
                nc.scalar.activation(skr[:, :wdt], ssk[:, :wdt], AF.Sqrt,
                                     scale=1.0, bias=epsk_sb)
                with nc.allow_low_precision(reason="k rms scale, bf16 ok"):
                    nc.vector.reciprocal(out=skrec_sb[sg][:, :wdt],
                                         in_=skr[:, :wdt])

            # -------- scale kh by sk/8, then AllGather 1 (overlaps V) ----
            TOTA = KC * 128 * W
            TOTB = n_half * 128 * VA_W
            groups = [[2 * x, 2 * x + 1] for x in range(N_CORES // 2)]
            blobA_loc = dp.tile([TOTA], bf16, name="blobA_loc")
            blobA_g = dp.tile([2 * TOTA], bf16, name="blobA_g")
            for m in range(KC):
                bcsk = po.tile([128, W], fp32, name="bcsk", tag="pv")
                for sg in range(n_hg):
                    wdt = min(512, W - sg * 512)
                    nc.tensor.matmul(
                        bcsk[:, sg * 512:sg * 512 + wdt],
                        lhsT=ind2c(m), rhs=skrec_sb[sg][:, :wdt],
                        start=True, stop=True)
                nc.vector.tensor_mul(khh_big[:, m * W:(m + 1) * W],
                                     khh_big[:, m * W:(m + 1) * W], bcsk)
            # p-major blob: (p, m, x)
            nc.sync.dma_start(
                out=blobA_loc.rearrange("(p x) -> p x", p=128),
                in_=khh_big[:, :])
            nc.gpsimd.collective_compute(
                "AllGather", mybir.AluOpType.bypass, replica_groups=groups,
                ins=[blobA_loc.opt()], outs=[blobA_g.opt()])

            # -------- V projection on the LOCAL kv half (ones-augmented) ----
            val_big = vp.tile([128, n_half * VA_W], bf16, name="val",
                              tag="val")
            for i in range(n_half):
                va = val_big[:, i * VA_W:(i + 1) * VA_W]
                ones_cols = bass.AP(tensor=va.tensor, offset=va.offset + 64,
                                    ap=[list(va.ap[0]), [65, H], [1, 1]])
                nc.vector.memset(ones_cols, 1.0)
                ps = pa.tile([128, 1024], fp32, name="proj_ps", tag="pa")
                for k in range(KC):          # k outer: one LDW serves both jn
                    for jn in range(2):
                        nc.tensor.matmul(
                            ps[:, jn * 512:(jn + 1) * 512],
                            lhsT=kvtc(k)[:, i * 128:(i + 1) * 128],
                            rhs=wvc(k)[:, jn * 512:(jn + 1) * 512],
                            start=(k == 0), stop=(k == KC - 1))
                for jn in range(2):
                    dst = bass.AP(tensor=va.tensor,
                                  offset=va.offset + 65 * 8 * jn,
                                  ap=[list(va.ap[0]), [65, 8], [1, 64]])
                    nc.vector.tensor_copy(
                        dst, ps[:, jn * 512:(jn + 1) * 512]
                        .rearrange("p (h d) -> p h d", h=8))
                nc.vector.tensor_scalar_mul(va, va, mask01_sb[:, i:i + 1])

            # -------- AllGather 2: va (overlaps the Q projection) ----
            blobB_loc = dp.tile([TOTB], bf16, name="blobB_loc")
            blobB_g = dp.tile([2 * TOTB], bf16, name="blobB_g")
            nc.sync.dma_start(
                out=blobB_loc.rearrange("(p x) -> p x", p=128),
                in_=val_big[:, :])
            nc.gpsimd.collective_compute(
                "AllGather", mybir.AluOpType.bypass, replica_groups=groups,
                ins=[blobB_loc.opt()], outs=[blobB_g.opt()])

            # ---------------- Q projection + q RMS stats ----------------
            # sumsq matmuls are emitted one (m, j) step late so the PE
            # queue head never blocks on the ACT Square.
            qh_sb = [qhp.tile([128, LC], bf16, name=f"qh{m}", tag="qh")
                     for m in range(KC)]
            sumsq_q = [po.tile([16, 512], fp32, name=f"ssq{j}", tag="pv")
                       for j in range(2)]
            pend = None                      # (m, qsq_tile)
            for m in range(KC):
                ps = pa.tile([128, 1024], fp32, name="proj_ps", tag="pa")
                for k in range(KC):          # k outer: one LDW serves both j
                    for j in range(2):
                        nc.tensor.matmul(
                            ps[:, j * 512:(j + 1) * 512],
                            lhsT=wqc(k)[:, m * 128:(m + 1) * 128],
                            rhs=qtc(k)[:, j * 512:(j + 1) * 512],
                            start=(k == 0), stop=(k == KC - 1))
                if pend is not None:
                    pm, pq = pend
                    for j in range(2):
                        nc.tensor.matmul(
                            sumsq_q[j][:, :], lhsT=indc(pm),
                            rhs=pq[:, j * 512:(j + 1) * 512],
                            start=(pm == 0), stop=(pm == KC - 1))
                nc.vector.tensor_scalar_mul(qh_sb[m][:, :], ps[:, :], qw_sb)
                qsq = kvp.tile([128, 1024], bf16, name="sqt", tag="sq",
                               bufs=6)
                nc.scalar.activation(qsq, ps, AF.Square)
                pend = (m, qsq)
            pm, pq = pend
            for j in range(2):
                nc.tensor.matmul(sumsq_q[j][:, :], lhsT=indc(pm),
                                 rhs=pq[:, j * 512:(j + 1) * 512],
                                 start=False, stop=True)
            # sq = 1/sqrt(mean + eps); broadcast with bf16 rank-1 matmuls
            sq_sb = []
            for j in range(2):
                sqr = tp.tile([16, 512], fp32, name=f"sqr{j}", tag="small16",
                              bufs=2)
                nc.scalar.activation(sqr, sumsq_q[j][:, :], AF.Sqrt,
                                     scale=1.0 / HD, bias=epsq_sb)
                sqv = tp.tile([16, 512], bf16, name=f"sqv{j}", tag="small16b",
                              bufs=2)
                with nc.allow_low_precision(reason="rank-1 rms scale, bf16"):
                    nc.vector.reciprocal(out=sqv, in_=sqr)
                sq_sb.append(sqv)
            for m in range(KC):
                for j in range(2):
                    bc = po.tile([128, 512], fp32, name="qbc", tag="pv")
                    nc.tensor.matmul(bc, lhsT=ind2c(m), rhs=sq_sb[j],
                                     start=True, stop=True)
                    nc.vector.tensor_mul(
                        qh_sb[m][:, j * 512:(j + 1) * 512],
                        qh_sb[m][:, j * 512:(j + 1) * 512], bc)

            # -------- readback into canonical full-S tiles --------
            kh_big = khp.tile([128, KC * 2 * W], bf16, name="kh", tag="kh")
            for r in range(2):
                nc.sync.dma_start(
                    out=kh_big.rearrange("p (m rx) -> p m rx", m=KC)
                    [:, :, r * W:(r + 1) * W],
                    in_=blobA_g[r * TOTA:(r + 1) * TOTA]
                    .rearrange("(p m x) -> p m x", p=128, m=KC))
            va_big = vp.tile([128, 2 * n_half * VA_W], bf16, name="vab",
                             tag="vab")
            for r in range(2):
                nc.sync.dma_start(
                    out=va_big[:, r * TOTB // 128:(r + 1) * TOTB // 128],
                    in_=blobB_g[r * TOTB:(r + 1) * TOTB]
                    .rearrange("(p y) -> p y", p=128))

            def khc(m):
                return kh_big[:, m * 2 * W:(m + 1) * 2 * W]

            def vac(i):
                return va_big[:, i * VA_W:(i + 1) * VA_W]

            # wo fetch now: reuses the wk slot (idle since K proj), lands
            # during attention.
            wo_big = wp.tile([128, KC * DIM], bf16, name="wo", tag="wbig")
            nc.sync.dma_start(out=wo_big, in_=wo_d[:, :])

            def woc(k):
                return wo_big[:, k * DIM:(k + 1) * DIM]

            # ------------- attention: one head at a time, ACT-paced -------
            # pv emitted TWO chunks late so it never waits on the exp; the
            # PE keeps a 2-deep score pipeline in the other direction.
            oT_big = qtp.tile([128, KC * LC], bf16, name="oT", tag="qt")

            def oTc(m):
                return oT_big[:, m * LC:(m + 1) * LC]

            den_sb = sp.tile([16, LC], fp32, name="den")
            nc.vector.memset(den_sb, 1.0)    # not-yet-written rows

            def emit_pv(pe):
                ph, pi, pex, ppv, first, last = pe
                for j in range(2):
                    nc.tensor.matmul(
                        ppv[:65, j * 512:(j + 1) * 512],
                        lhsT=vac(pi)[:, ph * 65:(ph + 1) * 65],
                        rhs=pex[:, j * 512:(j + 1) * 512],
                        start=first, stop=last)
                if last:
                    m, r = ph // 2, (ph % 2) * 64
                    dstage = tp.tile([128, LC], fp32, name="dstage",
                                     tag="rec", bufs=2)
                    nc.vector.tensor_copy(dstage[64:65, :], ppv[64:65, :])
                    nc.sync.dma_start(out=den_sb[ph:ph + 1, :],
                                      in_=dstage[64:65, :])
                    nc.vector.tensor_copy(oTc(m)[r:r + 64, :], ppv[0:64, :])

            pending = deque()
            for h in range(H):
                m, r = h // 2, (h % 2) * 64
                pv = po.tile([128, LC], fp32, name=f"pv{h}", tag="pv")
                for i in range(n_sc):
                    sc = pa.tile([128, LC], fp32, name="sc", tag="pa")
                    for j in range(2):
                        nc.tensor.matmul(
                            sc[:, j * 512:(j + 1) * 512],
                            lhsT=khc(m)[r:r + 64, i * 128:(i + 1) * 128],
                            rhs=qh_sb[m][r:r + 64, j * 512:(j + 1) * 512],
                            start=True, stop=True)
                    if len(pending) == 2:
                        emit_pv(pending.popleft())
                    ex = kvp.tile([128, LC], bf16, name="ex", tag="sq",
                                  bufs=6)
                    nc.scalar.activation(ex, sc, AF.Exp)
                    pending.append((h, i, ex, pv, i == 0, i == n_sc - 1))
            while pending:
                emit_pv(pending.popleft())

            # ------- normalize o^T + output projection, interleaved -------
            denr32 = tp.tile([16, LC], fp32, name="denr32", tag="rec",
                             bufs=2)
            nc.vector.reciprocal_approx_fast(out=denr32, in_=den_sb)
            denr_sb = sp.tile([16, LC], bf16, name="denr")
            nc.vector.tensor_copy(denr_sb, denr32)
            for j in range(2):
                for m in range(KC):
                    obc = po.tile([128, 512], fp32, name="obc", tag="pv")
                    nc.tensor.matmul(obc, lhsT=ind2c(m),
                                     rhs=denr_sb[:, j * 512:(j + 1) * 512],
                                     start=True, stop=True)
                    nc.vector.tensor_mul(
                        oTc(m)[:, j * 512:(j + 1) * 512],
                        oTc(m)[:, j * 512:(j + 1) * 512], obc)
                for lc in range(4 * j, 4 * j + 4):
                    for jn in range(2):
                        ps = pa.tile([128, 1024], fp32, name="proj_ps",
                                     tag="pa")
                        for k in range(KC):
                            nc.tensor.matmul(
                                ps[:, :512],
                                lhsT=oTc(k)[:, lc * 128:(lc + 1) * 128],
                                rhs=woc(k)[:, jn * 512:(jn + 1) * 512],
                                start=(k == 0), stop=(k == KC - 1))
                        osb = tp.tile([128, 512], fp32, name="osb", tag="rec",
                                      bufs=2)
                        nc.vector.tensor_copy(osb, ps[:, :512])
                        nc.sync.dma_start(
                            out=out_d[lc * 128:(lc + 1) * 128,
                                      jn * 512:(jn + 1) * 512],
                            in_=osb)
    nc.compile()
    return nc


def kernel(**inputs):
    q = np.asarray(inputs["q"], dtype=np.float32)
    kv = np.asarray(inputs["kv"], dtype=np.float32)
    seqlens = np.asarray(inputs["x_seqlens"], dtype=np.int32)
    Wq = np.asarray(inputs["Wq"], dtype=np.float32)
    Wk = np.asarray(inputs["Wk"], dtype=np.float32)
    Wv = np.asarray(inputs["Wv"], dtype=np.float32)
    Wo = np.asarray(inputs["Wo"], dtype=np.float32)
    qnw = np.asarray(inputs["q_norm_w"], dtype=np.float32)
    knw = np.asarray(inputs["k_norm_w"], dtype=np.float32)

    n_sc = max(1, int(-(-int(seqlens.max()) // 128)))
    if n_sc not in _CACHE:
        _CACHE[n_sc] = _build(n_sc)
    nc = _CACHE[n_sc]

    def pshuf(w):                 # [KC*128, X] -> [128, KC*X]
        w = np.asarray(w, dtype=np.float32)
        kc, x = w.shape[0] // 128, w.shape[1]
        return np.ascontiguousarray(
            w.reshape(kc, 128, x).transpose(1, 0, 2).reshape(128, kc * x)
        ).astype(BF16)

    wq_b, wk_b = pshuf(Wq), pshuf(Wk)
    wv_b, wo_b = pshuf(Wv), pshuf(Wo)
    qw = np.tile(qnw, 2).reshape(128, 1)
    kw = np.tile(knw, 2).reshape(128, 1)
    ind = np.zeros((KC, 128, 16), np.float32)
    ind2 = np.zeros((KC, 16, 128), np.float32)
    p = np.arange(128)
    for c in range(KC):
        ind[c, p, 2 * c + p // 64] = 1.0
        ind2[c, 2 * c + p // 64, p] = 1.0
    ind = np.ascontiguousarray(
        ind.transpose(1, 0, 2).reshape(128, KC * 16)).astype(BF16)
    ind2 = np.ascontiguousarray(
        ind2.transpose(1, 0, 2).reshape(16, KC * 128)).astype(BF16)

    in_maps = []
    for c in range(N_CORES):
        b, half = c // 2, c % 2
        qT = pshuf(q[b, half * LC:(half + 1) * LC, :].T)
        n_half = (n_sc + 1) // 2
        Wl = n_half * 128
        kvT = pshuf(kv[b].T[:, half * Wl:(half + 1) * Wl])
        sl = int(seqlens[b])
        gpos = half * Wl + np.arange(Wl)          # local kv global positions
        m01 = (gpos < sl).astype(np.float32).reshape(n_half, 128).T
        mask01 = np.zeros((128, 8), np.float32)
        mask01[:, :n_half] = m01
        in_maps.append({
            "qT": qT, "kvT": kvT, "wq": wq_b, "wk": wk_b, "wv": wv_b,
            "wo": wo_b, "mask01": mask01, "qw": qw, "kw": kw, "ind": ind,
            "ind2": ind2,
        })

    res = run_bass_kernel_spmd(nc, in_maps, list(range(N_CORES)),
                               trace=TRACE)
    LAST_RESULT["exec_time_ns"] = res.exec_time_ns
    LAST_RESULT["profile"] = res.profile_json

    out = np.empty((B, L, DIM), np.float32)
    for c in range(N_CORES):
        b, half = c // 2, c % 2
        out[b, half * LC:(half + 1) * LC, :] = res.results[c]["out"]
    return out
